# revision 1
# baseline (speedup 1.0000x reference)
"""Trainium2 Bass kernel for nn_AttentionLayer (B=4, S=2048, H=16, DH=64).

Sharding: 8 cores = 4 batches x 2 head-halves. Core c handles batch c//2,
heads (c%2)*8 .. (c%2)*8+8 (512 of the 1024 QKV columns).

Per-core structure (ACT-bound design; modeled ~287.4us, ACT busy ~257.6us):
  - All inputs arrive as bf16, host-prearranged so every DMA is contiguous
    per partition (strided gathers pay ~2x on the serial DMA device).
  - Q/K projections (PE, bf16, per head-pair column chunk) write q/k
    transposed via the DVE bias-add: qt/kt[m] [128p = 2 heads x 64 dh, S].
    The first pair-0 unit interleaves k and q matmuls per x-descriptor so
    exp0 fires ~11us in. V projection -> vt[kb] [128 kpos, 8 heads, 65]
    bf16 with col 64 = 1.0 (PV accumulates the softmax denominator free).
  - Attention stream: 512 slots (group = (head, 512-q block), h-major order
    so consecutive h0/h1 reuse pair-0 tiles and each pair's ~60us window
    hides the next pair's projections). The first two groups (h0/h1 at qb0)
    interleave per k-block so startup consumption matches the serial-DMA
    x-chunk delivery rate; group (1,0)'s PVs are deferred through the E
    ring until (0,0)'s norm frees the single ctx bank. Slots pack into alternating
    3-slot/2-slot PSUM score tiles ([2,2] prefix); one exp (ACT) per tile
    (1536/1024 wide) is the metronome. fp8 DoubleRow scores measured 2%
    rel err (over the 2e-2 gate) -- bf16 q,k (~0.3%) is used instead.
  - PV reoriented to ctx[q, d]: out [128 q, 65] per (slot, qtile), bf16
    E x V at 1.0 cycles/row, accumulated over kb into one memset-zeroed
    PSUM bank per group (start=False + skip_group_check lets 4 sub-bank
    accumulation groups share the bank; a start=True would zero the whole
    2KB zero-region).
  - Normalization: DVE reciprocal of ctx col 64 + ONE stride-0-broadcast
    tensor_mul (rr.broadcast_to), then a single DMA per group.
  - A ~2us stream of throwaway fp32 matmuls at t=0 ramps the PE p-state
    (full clock needs ~3us of continuous execution) during the DMA phase.
  PSUM banks: qkv 2 + scores 3+2 + ctx 1 = 8. Emission order defines Tile
  dependencies, so all projections are emitted (low-priority fill band)
  before the attention stream; band priorities interleave them at schedule
  time. Modeled: ACT 257.6us busy, PE ~249us busy, gaps ~27us
  (9 startup DMA-bound + 5 tail + scattered PE-saturation transients).
"""

import numpy as np

B, S, H, DH = 4, 2048, 16, 64
D = H * DH  # 1024
NCORES = 8
COLS = 512  # qkv columns per core (8 heads)
NKB = 16
EXP_SCALE = 0.125  # 1/sqrt(DH)

_CACHE = {}


def _build():
    import concourse.mybir as mybir
    import concourse.tile as tile
    from concourse import bacc

    f32 = mybir.dt.float32
    f32r = mybir.dt.float32r
    bf16 = mybir.dt.bfloat16
    Exp = mybir.ActivationFunctionType.Exp

    nc = bacc.Bacc(
        "TRN2",
        target_bir_lowering=False,
        debug=False,
        enable_asserts=False,
        num_devices=NCORES,
    )

    xT_d = nc.dram_tensor("xT", [128, 4, 8, 512], bf16, kind="ExternalInput").ap()
    wq_d = nc.dram_tensor("wq", [128, 4, 8, 128], bf16, kind="ExternalInput").ap()
    wk_d = nc.dram_tensor("wk", [128, 4, 8, 128], bf16, kind="ExternalInput").ap()
    wv_d = nc.dram_tensor("wv", [128, 8, COLS], bf16, kind="ExternalInput").ap()
    bqk_d = nc.dram_tensor("bqk", [128, 8], f32, kind="ExternalInput").ap()
    bv_d = nc.dram_tensor("bv", [COLS], f32, kind="ExternalInput").ap()
    out_d = nc.dram_tensor("out", [S, COLS], f32, kind="ExternalOutput").ap()

    with tile.TileContext(nc) as tc:
        with (
            tc.tile_pool(name="consts", bufs=1) as consts,
            tc.tile_pool(name="wpool", bufs=1) as wpool,
            tc.tile_pool(name="qkp", bufs=1) as qkp,
            tc.tile_pool(name="vpool", bufs=1) as vpool,
            tc.tile_pool(name="xpool", bufs=1) as xpool,
            tc.tile_pool(name="epool", bufs=1) as epool,
            tc.tile_pool(name="opool", bufs=1) as opool,
            tc.tile_pool(name="psum", bufs=1, space="PSUM") as psum,
        ):
            from contextlib import contextmanager

            base = tc.cur_priority + 50
            att_cur = [base]
            fill_cur = [base + 8000]

            @contextmanager
            def band(cursor):
                off = tc.cur_priority - cursor[0]
                with tc.high_priority(offset=off):
                    yield
                    cursor[0] = tc.cur_priority

            # ---- constants ----
            with band(att_cur):
                warm = consts.tile([1, 1], f32)
                nc.vector.memset(warm, 0.0)
                nc.scalar.activation(warm, warm, Exp)  # pull ACT table load early
                # PE p-state warm-up: the tensor engine reaches full clock
                # only after ~3us of continuous execution. Run ~4us of
                # throwaway fp32 matmuls during the initial DMA window so the
                # first real projections start at full speed.
                wsrc = consts.tile([128, 128], f32, name="wsrc")
                nc.vector.memset(wsrc, 0.0)
                for _ in range(13):
                    wps = psum.tile([128, 4, 65], f32, tag="ctx", bufs=1, name="wps")
                    nc.tensor.matmul(
                        wps.rearrange("p t d -> p (t d)")[:, 0:64],
                        lhsT=wsrc,
                        rhs=wsrc[:, 0:64],
                        start=True,
                        stop=True,
                    )

            with band(fill_cur):
                bqk_t = consts.tile([128, 8], f32)
                bv_s = consts.tile([1, COLS], f32)
                bvb = consts.tile([128, COLS], f32)
                nc.gpsimd.dma_start(out=bqk_t, in_=bqk_d)
                bq_t = bqk_t[:, 0:4]
                bk_t = bqk_t[:, 4:8]
                nc.gpsimd.dma_start(out=bv_s, in_=bv_d[None, :])
                nc.gpsimd.partition_broadcast(bvb, bv_s)

                vt = [vpool.tile([128, 8, 65], bf16, name=f"vt{i}") for i in range(NKB)]
                for i in range(NKB):
                    nc.vector.memset(vt[i][:, :, 64:65], 1.0)

                wv_t = wpool.tile([128, 8, COLS], bf16, name="wv_t")

                # bf16 q/k transposed tiles per head pair m:
                # [128p = 2 heads x 64 dh, 2048 s]
                qt_b = [qkp.tile([128, S], bf16, name=f"qt{m}") for m in range(4)]
                kt_b = [qkp.tile([128, S], bf16, name=f"kt{m}") for m in range(4)]

            # ---- weight chunk ring (2 bufs per proj; quad1 reuses quad0's) ----
            wcur = {"q": {}, "k": {}}

            def load_w(proj, m, eng):
                w_d = wq_d if proj == "q" else wk_d
                wt = wpool.tile(
                    [128, 8, 128], bf16, tag=f"w{proj}", bufs=2, name=f"w{proj}{m}"
                )
                eng.dma_start(out=wt, in_=w_d[:, m, :, :])
                wcur[proj][m] = wt

            with band(fill_cur):
                # critical-path DMA order (all on the SP/HWDGE path; gpsimd
                # dma_start occupies the Pool engine ~1.1us per descriptor):
                # wk0, x0, wk1, x1, wq0, x2, wq1, x3, wv
                xt = []
                for c in range(4):
                    xc = xpool.tile([128, 8, 512], bf16, name=f"xt{c}")
                    xt.append(xc)

                def load_x(c):
                    for j0 in range(0, 8, 2):
                        nc.sync.dma_start(
                            out=xt[c][:, j0 : j0 + 2, :], in_=xT_d[:, c, j0 : j0 + 2, :]
                        )

                load_w("k", 0, nc.sync)
                load_w("q", 0, nc.sync)
                load_x(0)
                load_x(1)
                load_x(2)
                load_x(3)
                load_w("k", 1, nc.sync)
                load_w("q", 1, nc.sync)
                nc.sync.dma_start(out=wv_t, in_=wv_d)



            def proj_kq_fused(m, c):
                # k and q projections for pair m interleaved per x descriptor,
                # so both finish right after the last x chunk lands
                psk = psum.tile([128, 512], f32, tag="qkv", bufs=2, name="psk")
                psq = psum.tile([128, 512], f32, tag="qkv", bufs=2, name="psq2")
                for j in range(8):
                    nc.tensor.matmul(
                        psk, lhsT=wcur["k"][m][:, j, :], rhs=xt[c][:, j, :],
                        start=(j == 0), stop=(j == 7),
                    )
                    nc.tensor.matmul(
                        psq, lhsT=wcur["q"][m][:, j, :], rhs=xt[c][:, j, :],
                        start=(j == 0), stop=(j == 7),
                    )
                nc.vector.tensor_scalar_add(
                    kt_b[m][:, c * 512 : (c + 1) * 512], psk, bk_t[:, m : m + 1]
                )
                nc.vector.tensor_scalar_add(
                    qt_b[m][:, c * 512 : (c + 1) * 512], psq, bq_t[:, m : m + 1]
                )

            # ---- projection unit emitters (fill band) ----
            def proj_qk(proj, m, c):
                dst = qt_b[m] if proj == "q" else kt_b[m]
                bias_t = bq_t if proj == "q" else bk_t
                w = wcur[proj][m]
                ps = psum.tile([128, 512], f32, tag="qkv", bufs=2, name="psq")
                for j in range(8):
                    nc.tensor.matmul(
                        ps,
                        lhsT=w[:, j, :],
                        rhs=xt[c][:, j, :],
                        start=(j == 0),
                        stop=(j == 7),
                    )
                nc.vector.tensor_scalar_add(
                    dst[:, c * 512 : (c + 1) * 512], ps, bias_t[:, m : m + 1]
                )

            def proj_v(m, c, i):
                # V for head-pair m, s-chunk c, seq subchunk i -> vt[4c+i]
                ps = psum.tile([128, 512], f32, tag="qkv", bufs=2, name="psv")
                for j in range(8):
                    nc.tensor.matmul(
                        ps[:, 0:128],
                        lhsT=xt[c][:, j, i * 128 : (i + 1) * 128],
                        rhs=wv_t[:, j, m * 128 : (m + 1) * 128],
                        start=(j == 0),
                        stop=(j == 7),
                    )
                nc.vector.tensor_add(
                    vt[4 * c + i][:, 2 * m : 2 * m + 2, 0:64],
                    ps[:, 0:128].rearrange("p (h d) -> p h d", h=2),
                    bvb[:, m * 128 : (m + 1) * 128].rearrange("p (h d) -> p h d", h=2),
                )

            # projection emission order: priority mirrors consumption
            # (h-major groups: pair m's k/q before pair m's first head).
            proj_order = []
            for m in range(4):
                if m >= 2:
                    proj_order += [("wl", "k", m), ("wl", "q", m)]
                proj_order += [("kq", m, 0), ("k", m, 1), ("q", m, 1)]
                proj_order += [("k", m, 2), ("q", m, 2), ("k", m, 3), ("q", m, 3)]
                proj_order += [("v", m, c, i) for c in range(4) for i in range(4)]

            def emit_proj_all():
                with band(fill_cur):
                    for u in proj_order:
                        if u[0] == "wl":
                            load_w(u[1], u[2], nc.sync)
                        elif u[0] == "v":
                            proj_v(u[1], u[2], u[3])
                        elif u[0] == "kq":
                            proj_kq_fused(u[1], u[2])
                        else:
                            proj_qk(u[0], u[1], u[2])

            # ---- attention stream ----
            # group order: h-major. Each head's 4 q-blocks run consecutively;
            # h0/h1 share pair-0 k/q tiles, so the 21 units of h1 need no new
            # projections -- that window hides pair-1's projections, etc.
            groups = [(h, qb) for h in range(8) for qb in range(4)]
            # interleave (h0,qb0) and (h1,qb0): they share all pair-0 tiles,
            # so consuming both per k-block matches the serial-DMA delivery
            # rate of x chunks during startup (no ACT stalls). Group (1,0)'s
            # PVs are deferred via the E ring until (0,0)'s norm frees the
            # single ctx bank.
            slots = []
            for kb in range(NKB):
                slots.append((0, 0, kb))
                slots.append((1, 0, kb))
            slots += [
                (h, qb, kb)
                for (h, qb) in groups
                if (h, qb) not in ((0, 0), (1, 0))
                for kb in range(NKB)
            ]
            units = []
            pos = 0
            ui = 0
            prefix = [2, 2]
            while pos < len(slots):
                if ui < len(prefix):
                    w = prefix[ui]
                else:
                    w = 3 if ui % 2 == 0 else 2
                w = min(w, len(slots) - pos)
                units.append(slots[pos : pos + w])
                pos += w
                ui += 1

            ctx_cur = [None]

            def emit_scores(u):
                unit = units[u]
                full = 3 if u % 2 == 0 else 2
                tag = "scA" if u % 2 == 0 else "scB"
                with band(att_cur):
                    sc = psum.tile([128, full, 512], f32, tag=tag, bufs=1, name="sc")
                    for i, (h, qb, kb) in enumerate(unit):
                        m, p0 = h // 2, 64 * (h % 2)
                        nc.tensor.matmul(
                            sc[:, i, :],
                            lhsT=kt_b[m][p0 : p0 + 64, kb * 128 : (kb + 1) * 128],
                            rhs=qt_b[m][p0 : p0 + 64, qb * 512 : (qb + 1) * 512],
                            start=True,
                            stop=True,
                        )
                return sc

            def emit_exp(u, sc):
                w = len(units[u])
                with band(att_cur):
                    ee = epool.tile([128, 3, 512], bf16, tag="e", bufs=34, name="ee")
                    nc.scalar.activation(
                        ee[:, 0:w, :], sc[:, 0:w, :], Exp, scale=EXP_SCALE
                    )
                return ee

            def do_pv(ee, i, h, qb, kb):
                ctx = ctx_cur[0]
                for qt in range(4):
                    nc.tensor.matmul(
                        ctx[:, qt, :],
                        lhsT=ee[:, i, qt * 128 : (qt + 1) * 128],
                        rhs=vt[kb][:, h, :],
                        start=False,
                        stop=False,
                        skip_group_check=True,
                    )

            def do_norm(h, qb):
                ctx = ctx_cur[0]
                rr = opool.tile([128, 4, 1], f32, tag="r", bufs=3, name="rr")
                nc.vector.reciprocal(rr, ctx[:, :, 64:65])
                ob = opool.tile([128, 4, 64], f32, tag="o", bufs=3, name="ob")
                nc.vector.tensor_mul(
                    ob, ctx[:, :, 0:64], rr.broadcast_to([128, 4, 64])
                )
                nc.sync.dma_start(
                    out=out_d[
                        qb * 512 : (qb + 1) * 512, h * 64 : (h + 1) * 64
                    ].rearrange("(t p) d -> p t d", p=128),
                    in_=ob,
                )

            deferred = []

            def emit_pv(u, ee):
                unit = units[u]
                with band(att_cur):
                    for i, (h, qb, kb) in enumerate(unit):
                        if (h, qb) == (1, 0):
                            # deferred startup-interleave groups: stash; E
                            # tiles stay alive in the deep ring. Flush each
                            # group when its kb15 arrives (slot order ends
                            # (0,0,15),(1,0,15),(0,1,15) so the ctx bank
                            # frees sequentially).
                            deferred.append((ee, i, h, qb, kb))
                            if kb == NKB - 1:
                                ctx_cur[0] = psum.tile(
                                    [128, 4, 65], f32, tag="ctx", bufs=1, name="ctx"
                                )
                                nc.vector.memset(ctx_cur[0], 0.0)
                                mine = [d for d in deferred if d[2] == h and d[3] == qb]
                                for dee, di, dh, dqb, dkb in mine:
                                    do_pv(dee, di, dh, dqb, dkb)
                                do_norm(h, qb)
                            continue
                        if kb == 0:
                            ctx_cur[0] = psum.tile(
                                [128, 4, 65], f32, tag="ctx", bufs=1, name="ctx"
                            )
                            nc.vector.memset(ctx_cur[0], 0.0)
                        do_pv(ee, i, h, qb, kb)
                        if kb == NKB - 1:
                            do_norm(h, qb)

            emit_proj_all()
            NU = len(units)
            scs = {0: emit_scores(0), 1: emit_scores(1)}
            for u in range(NU):
                ee = emit_exp(u, scs.pop(u))
                if u + 2 < NU:
                    scs[u + 2] = emit_scores(u + 2)
                emit_pv(u, ee)

    nc.compile()
    return nc


def _get_nc():
    if "nc" not in _CACHE:
        _CACHE["nc"] = _build()
    return _CACHE["nc"]


def _in_maps(x, Wq, bq, Wk, bk, Wv, bv):
    import ml_dtypes

    bf = ml_dtypes.bfloat16
    x = np.asarray(x, np.float32)
    maps = []
    for c in range(NCORES):
        b, hh = c // 2, c % 2
        cs = slice(hh * COLS, (hh + 1) * COLS)
        def warr(W):
            # [1024, 512] -> [128 p, 4 m, 8 j, 128 c]
            a = np.asarray(W, np.float32)[:, cs].astype(bf)
            return np.ascontiguousarray(
                a.reshape(8, 128, 4, 128).transpose(1, 2, 0, 3)
            )

        xTr = x[b].T.astype(bf).reshape(8, 128, 4, 512).transpose(1, 2, 0, 3)
        wvr = np.asarray(Wv, np.float32)[:, cs].astype(bf).reshape(8, 128, 512)
        maps.append(
            {
                "xT": np.ascontiguousarray(xTr),
                "wq": warr(Wq),
                "wk": warr(Wk),
                "wv": np.ascontiguousarray(wvr.transpose(1, 0, 2)),
                "bqk": np.ascontiguousarray(
                    np.concatenate(
                        [
                            np.asarray(bq, np.float32)[cs].reshape(4, 128).T,
                            np.asarray(bk, np.float32)[cs].reshape(4, 128).T,
                        ],
                        axis=1,
                    )
                ),
                "bv": np.ascontiguousarray(np.asarray(bv, np.float32)[cs]),
            }
        )
    return maps


def _run(inputs, trace=False):
    from concourse import bass_utils

    nc = _get_nc()
    res = bass_utils.run_bass_kernel_spmd(
        nc,
        _in_maps(**inputs),
        core_ids=list(range(NCORES)),
        trace=trace,
    )
    out = np.empty((B, S, D), np.float32)
    for c in range(NCORES):
        b, hh = c // 2, c % 2
        out[b, :, hh * COLS : (hh + 1) * COLS] = res.results[c]["out"]
    return out, res


def kernel(**inputs):
    out, _ = _run(inputs, trace=False)
    return out


if __name__ == "__main__":
    _get_nc()
    print("build ok")



# revision 2
# speedup vs baseline: 1.1181x; 1.1181x over previous
"""Trainium2 Bass kernel for nn_AttentionLayer (B=4, S=2048, H=16, DH=64).

Sharding: 8 cores = 4 batches x 2 head-halves. Core c handles batch c//2,
heads (c%2)*8 .. (c%2)*8+8 (512 of the 1024 QKV columns).

Design (v2, ~PE-bound at ~194us modeled):
  - Projections (PE, bf16): x[2048,1024] @ W[1024,512] per head-pair chunk,
    as in the baseline. Q/K write TRANSPOSED via the DVE epilogue.
  - Scores in fp8e4m3 with DoubleRow (0.5 cycles/row, halves score PE time):
    kt8[m] [128p=2 heads x 64dh, S] fp8 holds k8; qt8[m] [128, 2, S] holds
    (q8, qres8 = q - q8). The DoubleRow matmul contracts
    k8^T q8 + k8^T qres8 = k8^T q  -- the q-side fp8 quantization error is
    residual-corrected for free using the second k-tile slot (the k8 operand
    is a stride-0 broadcast AP, verified on HW). Only the k-side fp8 error
    (~2.5% elementwise) survives -> ~1% on probs.
  - exp is split between ACT and DVE. ACT units: activation(Exp, bf16 out).
    DVE units: ONE tensor_scalar t = s*A + B with int16 output, where
    A = 128*log2(e)/8 and B = 16256 - 128*0.0573. Round-to-nearest convert
    makes the int16 bit pattern EXACTLY the Schraudolph bf16 approximation
    of exp(s/8) (verified bit-exact on HW); PV reads the tile bitcast as
    bf16. ~1.7% RMS elementwise on the offloaded fraction only.
  - Score units are uniform 2-slot tiles (tag sc, [128,2,512], bufs=2 = 4
    PSUM banks) -> one exp instruction per 2 slots on either engine.
  - ctx accumulates per (head, qb) group in PSUM with bufs=2 (2 banks), so
    group boundaries do not serialize PE through the DVE norm. The first PV
    matmul of a group uses start=True to zero the whole bank (replaces the
    DVE memset). V tiles carry a ones-column so PV accumulates the softmax
    denominator in ctx col 64; norm = DVE reciprocal + broadcast mul.
  - PSUM banks: qkv 2 + sc 4 + ctx 2 = 8.
  - Startup: DMA order wk0,wq0,wv0,x0(4),x1(2),wv1,x2,x3,wv2,wv3,wk1,wq1
    (wv is loaded in per-pair chunks from a [4,128,1024] host layout so V
    for pair 0 is available right after x0). ~13 throwaway fp32 matmuls
    ramp the PE p-state during the DMA window.
  - The last group's norm/DMA is split in halves to shorten the tail.
"""

import numpy as np

B, S, H, DH = 4, 2048, 16, 64
D = H * DH  # 1024
NCORES = 8
COLS = 512  # qkv columns per core (8 heads)
NKB = 16
EXP_SCALE = 0.125  # 1/sqrt(DH)
LOG2E = 1.4426950408889634
A_EXP = 128.0 * LOG2E * 0.125
B_EXP = 16256.0 - 128.0 * 0.0573  # bf16 exp bias + Schraudolph centering

_CACHE = {}


def _build(n_dve_mod=(4, 3, 2)):
    import concourse.mybir as mybir
    import concourse.tile as tile
    from concourse import bacc

    f32 = mybir.dt.float32
    bf16 = mybir.dt.bfloat16
    fp8 = mybir.dt.float8e4
    i16 = mybir.dt.int16
    Alu = mybir.AluOpType
    Exp = mybir.ActivationFunctionType.Exp
    DR = mybir.MatmulPerfMode.DoubleRow

    nc = bacc.Bacc(
        "TRN2",
        target_bir_lowering=False,
        debug=False,
        enable_asserts=False,
        num_devices=NCORES,
    )

    xT_d = nc.dram_tensor("xT", [128, 4, 8, 512], bf16, kind="ExternalInput").ap()
    wq_d = nc.dram_tensor("wq", [128, 4, 8, 128], bf16, kind="ExternalInput").ap()
    wk_d = nc.dram_tensor("wk", [128, 4, 8, 128], bf16, kind="ExternalInput").ap()
    wv_d = nc.dram_tensor("wv", [4, 128, 1024], bf16, kind="ExternalInput").ap()
    bqk_d = nc.dram_tensor("bqk", [128, 8], f32, kind="ExternalInput").ap()
    bv_d = nc.dram_tensor("bv", [COLS], f32, kind="ExternalInput").ap()
    out_d = nc.dram_tensor("out", [S, COLS], f32, kind="ExternalOutput").ap()

    with tile.TileContext(nc) as tc:
        with (
            tc.tile_pool(name="consts", bufs=1) as consts,
            tc.tile_pool(name="wpool", bufs=1) as wpool,
            tc.tile_pool(name="qkp", bufs=1) as qkp,
            tc.tile_pool(name="vpool", bufs=1) as vpool,
            tc.tile_pool(name="xpool", bufs=1) as xpool,
            tc.tile_pool(name="epool", bufs=1) as epool,
            tc.tile_pool(name="opool", bufs=1) as opool,
            tc.tile_pool(name="psum", bufs=1, space="PSUM") as psum,
        ):
            from contextlib import contextmanager

            base = tc.cur_priority + 50
            att_cur = [base]
            fill_cur = [base + 8000]

            @contextmanager
            def band(cursor):
                off = tc.cur_priority - cursor[0]
                with tc.high_priority(offset=off):
                    yield
                    cursor[0] = tc.cur_priority

            # ---- constants + PE p-state warm-up ----
            with band(att_cur):
                warm = consts.tile([1, 1], f32)
                nc.vector.memset(warm, 0.0)
                nc.scalar.activation(warm, warm, Exp)  # pull ACT table load
                wsrc = consts.tile([128, 128], f32, name="wsrc")
                nc.vector.memset(wsrc, 0.0)
                for _ in range(13):
                    wps = psum.tile([128, 4, 65], f32, tag="ctx", bufs=2, name="wps")
                    nc.tensor.matmul(
                        wps.rearrange("p t d -> p (t d)")[:, 0:64],
                        lhsT=wsrc,
                        rhs=wsrc[:, 0:64],
                        start=True,
                        stop=True,
                    )

            with band(fill_cur):
                bqk_t = consts.tile([128, 8], f32)
                bv_s = consts.tile([1, COLS], f32)
                bvb = consts.tile([128, COLS], f32)
                nc.gpsimd.dma_start(out=bqk_t, in_=bqk_d)
                bq_t = bqk_t[:, 0:4]
                bk_t = bqk_t[:, 4:8]
                nc.gpsimd.dma_start(out=bv_s, in_=bv_d[None, :])
                nc.gpsimd.partition_broadcast(bvb, bv_s)

                vt = [vpool.tile([128, 8, 65], bf16, name=f"vt{i}") for i in range(NKB)]
                for i in range(NKB):
                    nc.vector.memset(vt[i][:, :, 64:65], 1.0)

                # fp8 q/k tiles: kt8[m] [128p = 2 heads x 64 dh, S];
                # qt8[m] [128, 2, S] = (q8, qres8)
                kt8 = [qkp.tile([128, S], fp8, name=f"kt8{m}") for m in range(4)]
                qt8 = [qkp.tile([128, 2, S], fp8, name=f"qt8{m}") for m in range(4)]
                wvt = [wpool.tile([128, 8, 128], bf16, name=f"wvt{m}") for m in range(4)]

            # ---- weight chunk ring ----
            wcur = {"q": {}, "k": {}}

            def load_w(proj, m):
                w_d = wq_d if proj == "q" else wk_d
                wt = wpool.tile(
                    [128, 8, 128], bf16, tag=f"w{proj}", bufs=2, name=f"w{proj}{m}"
                )
                nc.sync.dma_start(out=wt, in_=w_d[:, m, :, :])
                wcur[proj][m] = wt

            def load_wv(m):
                nc.sync.dma_start(
                    out=wvt[m], in_=wv_d[m].rearrange("p (j c) -> p j c", j=8)
                )

            with band(fill_cur):
                xt = [
                    xpool.tile([128, 8, 512], bf16, name=f"xt{c}") for c in range(4)
                ]

                # DMA order: startup-critical first; x0 split fine for
                # matmul chasing, later chunks coarser.
                load_w("k", 0)
                load_w("q", 0)
                load_wv(0)
                for j0 in range(0, 8, 2):
                    nc.sync.dma_start(
                        out=xt[0][:, j0 : j0 + 2, :], in_=xT_d[:, 0, j0 : j0 + 2, :]
                    )
                for j0 in range(0, 8, 4):
                    nc.sync.dma_start(
                        out=xt[1][:, j0 : j0 + 4, :], in_=xT_d[:, 1, j0 : j0 + 4, :]
                    )
                load_wv(1)
                nc.sync.dma_start(out=xt[2], in_=xT_d[:, 2, :, :])
                nc.sync.dma_start(out=xt[3], in_=xT_d[:, 3, :, :])
                load_wv(2)
                load_wv(3)
                load_w("k", 1)
                load_w("q", 1)

            # ---- projection epilogues (DVE, PSUM -> fp8 SBUF) ----
            def epi_k(m, c, ps):
                ch = slice(c * 512, (c + 1) * 512)
                nc.vector.tensor_scalar_add(kt8[m][:, ch], ps, bk_t[:, m : m + 1])

            def epi_q(m, c, ps):
                ch = slice(c * 512, (c + 1) * 512)
                nc.vector.tensor_scalar_add(qt8[m][:, 0, ch], ps, bq_t[:, m : m + 1])
                nc.vector.scalar_tensor_tensor(
                    qt8[m][:, 1, ch],
                    ps,
                    bq_t[:, m : m + 1],
                    qt8[m][:, 0, ch],
                    Alu.add,
                    Alu.subtract,
                )

            # ---- projection unit emitters (fill band) ----
            def proj_kq_fused(m, c):
                psk = psum.tile([128, 512], f32, tag="qkv", bufs=2, name="psk")
                psq = psum.tile([128, 512], f32, tag="qkv", bufs=2, name="psq2")
                for j in range(8):
                    nc.tensor.matmul(
                        psk, lhsT=wcur["k"][m][:, j, :], rhs=xt[c][:, j, :],
                        start=(j == 0), stop=(j == 7),
                    )
                    nc.tensor.matmul(
                        psq, lhsT=wcur["q"][m][:, j, :], rhs=xt[c][:, j, :],
                        start=(j == 0), stop=(j == 7),
                    )
                epi_k(m, c, psk)
                epi_q(m, c, psq)

            def proj_qk(proj, m, c):
                w = wcur[proj][m]
                ps = psum.tile([128, 512], f32, tag="qkv", bufs=2, name="psq")
                for j in range(8):
                    nc.tensor.matmul(
                        ps,
                        lhsT=w[:, j, :],
                        rhs=xt[c][:, j, :],
                        start=(j == 0),
                        stop=(j == 7),
                    )
                (epi_q if proj == "q" else epi_k)(m, c, ps)

            def proj_v(m, c, i):
                ps = psum.tile([128, 512], f32, tag="qkv", bufs=2, name="psv")
                for j in range(8):
                    nc.tensor.matmul(
                        ps[:, 0:128],
                        lhsT=xt[c][:, j, i * 128 : (i + 1) * 128],
                        rhs=wvt[m][:, j, :],
                        start=(j == 0),
                        stop=(j == 7),
                    )
                nc.vector.tensor_add(
                    vt[4 * c + i][:, 2 * m : 2 * m + 2, 0:64],
                    ps[:, 0:128].rearrange("p (h d) -> p h d", h=2),
                    bvb[:, m * 128 : (m + 1) * 128].rearrange("p (h d) -> p h d", h=2),
                )

            proj_order = []
            for m in range(4):
                if m >= 2:
                    proj_order += [("wl", "k", m), ("wl", "q", m)]
                proj_order += [("kq", m, 0)]
                proj_order += [("k", m, 1), ("q", m, 1)]
                proj_order += [("k", m, 2), ("q", m, 2), ("k", m, 3), ("q", m, 3)]
                proj_order += [("v", m, c, i) for c in range(4) for i in range(4)]

            def emit_proj_all():
                with band(fill_cur):
                    for u in proj_order:
                        if u[0] == "wl":
                            load_w(u[1], u[2])
                        elif u[0] == "v":
                            proj_v(u[1], u[2], u[3])
                        elif u[0] == "kq":
                            proj_kq_fused(u[1], u[2])
                        else:
                            proj_qk(u[0], u[1], u[2])

            # ---- attention stream ----
            # groups h-major; the first two groups (h0/h1 at qb0) interleave
            # per k-block (each gets its own ctx bank via bufs=2).
            groups = [(h, qb) for h in range(8) for qb in range(4)]
            slots = []
            for kb in range(NKB):
                slots.append((0, 0, kb))
                slots.append((1, 0, kb))
            slots += [
                (h, qb, kb)
                for (h, qb) in groups
                if (h, qb) not in ((0, 0), (1, 0))
                for kb in range(NKB)
            ]
            # uniform 2-slot units
            units = [slots[p : p + 2] for p in range(0, len(slots), 2)]
            NU = len(units)

            # engine assignment: more DVE exp in the proj-free endgame
            def is_dve(u):
                if u < 16:
                    return u % n_dve_mod[0] == 2
                if u < 192:
                    return u % n_dve_mod[1] == 1
                return u % n_dve_mod[2] == 0

            def emit_scores(u):
                unit = units[u]
                with band(att_cur):
                    sc = psum.tile([128, 2, 512], f32, tag="sc", bufs=2, name="sc")
                    for i, (h, qb, kb) in enumerate(unit):
                        m, p0 = h // 2, 64 * (h % 2)
                        kb_ap = (
                            kt8[m][p0 : p0 + 64, kb * 128 : (kb + 1) * 128]
                            .rearrange("p (one f) -> p one f", one=1)
                            .broadcast_to([64, 2, 128])
                        )
                        nc.tensor.matmul(
                            sc[:, i, :],
                            lhsT=kb_ap,
                            rhs=qt8[m][p0 : p0 + 64, :, qb * 512 : (qb + 1) * 512],
                            start=True,
                            stop=True,
                            perf_mode=DR,
                        )
                return sc

            def emit_exp(u, sc):
                w = len(units[u])
                with band(att_cur):
                    if is_dve(u):
                        ed = epool.tile([128, 2, 512], i16, tag="ed", bufs=6, name="ed")
                        nc.vector.tensor_scalar(
                            ed[:, 0:w, :], sc[:, 0:w, :], A_EXP, B_EXP,
                            Alu.mult, Alu.add,
                        )
                        return ed.bitcast(mybir.dt.bfloat16)
                    ee = epool.tile([128, 2, 512], mybir.dt.bfloat16, tag="ee",
                                    bufs=8, name="ee")
                    nc.scalar.activation(
                        ee[:, 0:w, :], sc[:, 0:w, :], Exp, scale=EXP_SCALE
                    )
                    return ee

            ctx_of = {}

            def do_pv(ev, i, h, qb, kb):
                if kb == 0:
                    ctx_of[(h, qb)] = psum.tile(
                        [128, 4, 65], f32, tag="ctx", bufs=2, name="ctx"
                    )
                ctx = ctx_of[(h, qb)]
                for qt in range(4):
                    nc.tensor.matmul(
                        ctx[:, qt, :],
                        lhsT=ev[:, i, qt * 128 : (qt + 1) * 128],
                        rhs=vt[kb][:, h, :],
                        start=(kb == 0 and qt == 0),
                        stop=False,
                        skip_group_check=True,
                    )

            def do_norm(h, qb, t0, t1):
                ctx = ctx_of[(h, qb)]
                nt = t1 - t0
                rr = opool.tile([128, 4, 1], f32, tag="r", bufs=3, name="rr")
                nc.vector.reciprocal(rr[:, t0:t1], ctx[:, t0:t1, 64:65])
                ob = opool.tile([128, 4, 64], f32, tag="o", bufs=3, name="ob")
                nc.vector.tensor_mul(
                    ob[:, t0:t1], ctx[:, t0:t1, 0:64],
                    rr[:, t0:t1].broadcast_to([128, nt, 64]),
                )
                nc.sync.dma_start(
                    out=out_d[
                        qb * 512 + t0 * 128 : qb * 512 + t1 * 128,
                        h * 64 : (h + 1) * 64,
                    ].rearrange("(t p) d -> p t d", p=128),
                    in_=ob[:, t0:t1],
                )

            last_group = groups[-1]

            def emit_pv(u, ev):
                unit = units[u]
                with band(att_cur):
                    for i, (h, qb, kb) in enumerate(unit):
                        if kb == NKB - 1 and (h, qb) == last_group:
                            # split the final norm to shorten the tail
                            if (h, qb) not in ctx_of:
                                do_pv(ev, i, h, qb, kb)
                                do_norm(h, qb, 0, 4)
                                continue
                            ctx = ctx_of[(h, qb)]
                            for qt in range(4):
                                nc.tensor.matmul(
                                    ctx[:, qt, :],
                                    lhsT=ev[:, i, qt * 128 : (qt + 1) * 128],
                                    rhs=vt[kb][:, h, :],
                                    start=False,
                                    stop=False,
                                    skip_group_check=True,
                                )
                                if qt == 1:
                                    do_norm(h, qb, 0, 2)
                            do_norm(h, qb, 2, 4)
                            continue
                        do_pv(ev, i, h, qb, kb)
                        if kb == NKB - 1:
                            do_norm(h, qb, 0, 4)

            emit_proj_all()
            scs = {0: emit_scores(0), 1: emit_scores(1)}
            for u in range(NU):
                ev = emit_exp(u, scs.pop(u))
                if u + 2 < NU:
                    scs[u + 2] = emit_scores(u + 2)
                emit_pv(u, ev)

    nc.compile()
    return nc


def _get_nc():
    if "nc" not in _CACHE:
        _CACHE["nc"] = _build()
    return _CACHE["nc"]


def _in_maps(x, Wq, bq, Wk, bk, Wv, bv):
    import ml_dtypes

    bf = ml_dtypes.bfloat16
    x = np.asarray(x, np.float32)
    maps = []
    for c in range(NCORES):
        b, hh = c // 2, c % 2
        cs = slice(hh * COLS, (hh + 1) * COLS)

        def warr(W):
            # [1024, 512] -> [128 p, 4 m, 8 j, 128 c]
            a = np.asarray(W, np.float32)[:, cs].astype(bf)
            return np.ascontiguousarray(
                a.reshape(8, 128, 4, 128).transpose(1, 2, 0, 3)
            )

        xTr = x[b].T.astype(bf).reshape(8, 128, 4, 512).transpose(1, 2, 0, 3)
        # wv: [1024, 512] -> [4 m, 128 p, 8 j * 128 c]
        wvr = np.asarray(Wv, np.float32)[:, cs].astype(bf)
        wvr = wvr.reshape(8, 128, 4, 128).transpose(2, 1, 0, 3).reshape(4, 128, 1024)
        maps.append(
            {
                "xT": np.ascontiguousarray(xTr),
                "wq": warr(Wq),
                "wk": warr(Wk),
                "wv": np.ascontiguousarray(wvr),
                "bqk": np.ascontiguousarray(
                    np.concatenate(
                        [
                            np.asarray(bq, np.float32)[cs].reshape(4, 128).T,
                            np.asarray(bk, np.float32)[cs].reshape(4, 128).T,
                        ],
                        axis=1,
                    )
                ),
                "bv": np.ascontiguousarray(np.asarray(bv, np.float32)[cs]),
            }
        )
    return maps


def _run(inputs, trace=False):
    from concourse import bass_utils

    nc = _get_nc()
    res = bass_utils.run_bass_kernel_spmd(
        nc,
        _in_maps(**inputs),
        core_ids=list(range(NCORES)),
        trace=trace,
    )
    out = np.empty((B, S, D), np.float32)
    for c in range(NCORES):
        b, hh = c // 2, c % 2
        out[b, :, hh * COLS : (hh + 1) * COLS] = res.results[c]["out"]
    return out, res


def kernel(**inputs):
    out, _ = _run(inputs, trace=False)
    return out


if __name__ == "__main__":
    _get_nc()
    print("build ok")


# revision 3
# speedup vs baseline: 1.1582x; 1.0358x over previous
"""Trainium2 Bass kernel for nn_AttentionLayer (B=4, S=2048, H=16, DH=64).

Sharding: 8 cores = 4 batches x 2 head-halves. Core c handles batch c//2,
heads (c%2)*8 .. (c%2)*8+8 (512 of the 1024 QKV columns).

Design (v2, ~PE-bound at ~194us modeled):
  - Projections (PE, bf16): x[2048,1024] @ W[1024,512] per head-pair chunk,
    as in the baseline. Q/K write TRANSPOSED via the DVE epilogue.
  - Scores in fp8e4m3 with DoubleRow (0.5 cycles/row, halves score PE time):
    kt8[m] [128p=2 heads x 64dh, S] fp8 holds k8; qt8[m] [128, 2, S] holds
    (q8, qres8 = q - q8). The DoubleRow matmul contracts
    k8^T q8 + k8^T qres8 = k8^T q  -- the q-side fp8 quantization error is
    residual-corrected for free using the second k-tile slot (the k8 operand
    is a stride-0 broadcast AP, verified on HW). Only the k-side fp8 error
    (~2.5% elementwise) survives -> ~1% on probs.
  - exp is split between ACT and DVE. ACT units: activation(Exp, bf16 out).
    DVE units: ONE tensor_scalar t = s*A + B with int16 output, where
    A = 128*log2(e)/8 and B = 16256 - 128*0.0573. Round-to-nearest convert
    makes the int16 bit pattern EXACTLY the Schraudolph bf16 approximation
    of exp(s/8) (verified bit-exact on HW); PV reads the tile bitcast as
    bf16. ~1.7% RMS elementwise on the offloaded fraction only.
  - Score units are uniform 2-slot tiles (tag sc, [128,2,512], bufs=2 = 4
    PSUM banks) -> one exp instruction per 2 slots on either engine.
  - ctx accumulates per (head, qb) group in PSUM with bufs=2 (2 banks), so
    group boundaries do not serialize PE through the DVE norm. The first PV
    matmul of a group uses start=True to zero the whole bank (replaces the
    DVE memset). V tiles carry a ones-column so PV accumulates the softmax
    denominator in ctx col 64; norm = DVE reciprocal + broadcast mul.
  - PSUM banks: qkv 2 + sc 4 + ctx 2 = 8.
  - Startup: DMA order wk0,wq0,wv0,x0(4),x1(2),wv1,x2,x3,wv2,wv3,wk1,wq1
    (wv is loaded in per-pair chunks from a [4,128,1024] host layout so V
    for pair 0 is available right after x0). ~13 throwaway fp32 matmuls
    ramp the PE p-state during the DMA window.
  - The last group's norm/DMA is split in halves to shorten the tail.
"""

import numpy as np

B, S, H, DH = 4, 2048, 16, 64
D = H * DH  # 1024
NCORES = 8
COLS = 512  # qkv columns per core (8 heads)
NKB = 16
EXP_SCALE = 0.125  # 1/sqrt(DH)
LOG2E = 1.4426950408889634
A_EXP = 128.0 * LOG2E * 0.125
B_EXP = 16256.0 - 128.0 * 0.0573  # bf16 exp bias + Schraudolph centering

_CACHE = {}


def _build(n_dve_mod=(4, 3, 2)):
    import concourse.mybir as mybir
    import concourse.tile as tile
    from concourse import bacc

    f32 = mybir.dt.float32
    bf16 = mybir.dt.bfloat16
    fp8 = mybir.dt.float8e4
    i16 = mybir.dt.int16
    Alu = mybir.AluOpType
    Exp = mybir.ActivationFunctionType.Exp
    DR = mybir.MatmulPerfMode.DoubleRow

    nc = bacc.Bacc(
        "TRN2",
        target_bir_lowering=False,
        debug=False,
        enable_asserts=False,
        num_devices=NCORES,
    )

    xT_d = nc.dram_tensor("xT", [128, 4, 8, 512], bf16, kind="ExternalInput").ap()
    wq_d = nc.dram_tensor("wq", [128, 4, 8, 128], bf16, kind="ExternalInput").ap()
    wk_d = nc.dram_tensor("wk", [128, 4, 8, 128], bf16, kind="ExternalInput").ap()
    wv_d = nc.dram_tensor("wv", [4, 128, 1024], bf16, kind="ExternalInput").ap()
    bqk_d = nc.dram_tensor("bqk", [128, 8], f32, kind="ExternalInput").ap()
    bv_d = nc.dram_tensor("bv", [COLS], f32, kind="ExternalInput").ap()
    out_d = nc.dram_tensor("out", [S, COLS], f32, kind="ExternalOutput").ap()

    with tile.TileContext(nc) as tc:
        with (
            tc.tile_pool(name="consts", bufs=1) as consts,
            tc.tile_pool(name="wpool", bufs=1) as wpool,
            tc.tile_pool(name="qkp", bufs=1) as qkp,
            tc.tile_pool(name="vpool", bufs=1) as vpool,
            tc.tile_pool(name="xpool", bufs=1) as xpool,
            tc.tile_pool(name="epool", bufs=1) as epool,
            tc.tile_pool(name="opool", bufs=1) as opool,
            tc.tile_pool(name="psum", bufs=1, space="PSUM") as psum,
        ):
            from contextlib import contextmanager

            base = tc.cur_priority + 50
            att_cur = [base]
            fill_cur = [base + 8000]

            @contextmanager
            def band(cursor):
                off = tc.cur_priority - cursor[0]
                with tc.high_priority(offset=off):
                    yield
                    cursor[0] = tc.cur_priority

            # ---- constants + PE p-state warm-up ----
            with band(att_cur):
                warm = consts.tile([1, 1], f32)
                nc.vector.memset(warm, 0.0)
                nc.scalar.activation(warm, warm, Exp)  # pull ACT table load
                wsrc = consts.tile([128, 128], f32, name="wsrc")
                nc.vector.memset(wsrc, 0.0)
                for _ in range(13):
                    wps = psum.tile([128, 4, 65], f32, tag="ctx", bufs=2, name="wps")
                    nc.tensor.matmul(
                        wps.rearrange("p t d -> p (t d)")[:, 0:64],
                        lhsT=wsrc,
                        rhs=wsrc[:, 0:64],
                        start=True,
                        stop=True,
                    )

            with band(fill_cur):
                bqk_t = consts.tile([128, 8], f32)
                bv_s = consts.tile([1, COLS], f32)
                bvb = consts.tile([128, COLS], f32)
                nc.gpsimd.dma_start(out=bqk_t, in_=bqk_d)
                bq_t = bqk_t[:, 0:4]
                bk_t = bqk_t[:, 4:8]
                nc.gpsimd.dma_start(out=bv_s, in_=bv_d[None, :])
                nc.gpsimd.partition_broadcast(bvb, bv_s)

                vt = [vpool.tile([128, 8, 65], bf16, name=f"vt{i}") for i in range(NKB)]
                for i in range(NKB):
                    nc.vector.memset(vt[i][:, :, 64:65], 1.0)

                # fp8 q/k tiles: kt8[m] [128p = 2 heads x 64 dh, S];
                # qt8[m] [128, 2, S] = (q8, qres8)
                kt8 = [qkp.tile([128, S], fp8, name=f"kt8{m}") for m in range(4)]
                qt8 = [qkp.tile([128, 2, S], fp8, name=f"qt8{m}") for m in range(4)]
                wvt = [wpool.tile([128, 8, 128], bf16, name=f"wvt{m}") for m in range(4)]

            # ---- weight chunk ring ----
            wcur = {"q": {}, "k": {}}

            def load_w(proj, m):
                w_d = wq_d if proj == "q" else wk_d
                wt = wpool.tile(
                    [128, 8, 128], bf16, tag=f"w{proj}", bufs=2, name=f"w{proj}{m}"
                )
                nc.sync.dma_start(out=wt, in_=w_d[:, m, :, :])
                wcur[proj][m] = wt

            def load_wv(m):
                nc.sync.dma_start(
                    out=wvt[m], in_=wv_d[m].rearrange("p (j c) -> p j c", j=8)
                )

            with band(fill_cur):
                xt = [
                    xpool.tile([128, 8, 512], bf16, name=f"xt{c}") for c in range(4)
                ]

                # DMA order: startup-critical first; x0 split fine for
                # matmul chasing, later chunks coarser.
                load_w("k", 0)
                load_w("q", 0)
                load_wv(0)
                for j0 in range(0, 8, 2):
                    nc.sync.dma_start(
                        out=xt[0][:, j0 : j0 + 2, :], in_=xT_d[:, 0, j0 : j0 + 2, :]
                    )
                for j0 in range(0, 8, 4):
                    nc.sync.dma_start(
                        out=xt[1][:, j0 : j0 + 4, :], in_=xT_d[:, 1, j0 : j0 + 4, :]
                    )
                load_wv(1)
                nc.sync.dma_start(out=xt[2], in_=xT_d[:, 2, :, :])
                nc.sync.dma_start(out=xt[3], in_=xT_d[:, 3, :, :])
                load_wv(2)
                load_wv(3)
                load_w("k", 1)
                load_w("q", 1)

            # ---- projection epilogues (DVE, PSUM -> fp8 SBUF) ----
            def epi_k(m, c, ps):
                ch = slice(c * 512, (c + 1) * 512)
                nc.vector.tensor_scalar_add(kt8[m][:, ch], ps, bk_t[:, m : m + 1])

            def epi_q(m, c, ps):
                ch = slice(c * 512, (c + 1) * 512)
                nc.vector.tensor_scalar_add(qt8[m][:, 0, ch], ps, bq_t[:, m : m + 1])
                nc.vector.scalar_tensor_tensor(
                    qt8[m][:, 1, ch],
                    ps,
                    bq_t[:, m : m + 1],
                    qt8[m][:, 0, ch],
                    Alu.add,
                    Alu.subtract,
                )

            # ---- projection unit emitters (fill band) ----
            def proj_kq_fused(m, c):
                psk = psum.tile([128, 512], f32, tag="qkv", bufs=2, name="psk")
                psq = psum.tile([128, 512], f32, tag="qkv", bufs=2, name="psq2")
                for j in range(8):
                    nc.tensor.matmul(
                        psk, lhsT=wcur["k"][m][:, j, :], rhs=xt[c][:, j, :],
                        start=(j == 0), stop=(j == 7),
                    )
                    nc.tensor.matmul(
                        psq, lhsT=wcur["q"][m][:, j, :], rhs=xt[c][:, j, :],
                        start=(j == 0), stop=(j == 7),
                    )
                epi_k(m, c, psk)
                epi_q(m, c, psq)

            def proj_qk(proj, m, c):
                w = wcur[proj][m]
                ps = psum.tile([128, 512], f32, tag="qkv", bufs=2, name="psq")
                for j in range(8):
                    nc.tensor.matmul(
                        ps,
                        lhsT=w[:, j, :],
                        rhs=xt[c][:, j, :],
                        start=(j == 0),
                        stop=(j == 7),
                    )
                (epi_q if proj == "q" else epi_k)(m, c, ps)

            def proj_v(m, c, i):
                ps = psum.tile([128, 512], f32, tag="qkv", bufs=2, name="psv")
                for j in range(8):
                    nc.tensor.matmul(
                        ps[:, 0:128],
                        lhsT=xt[c][:, j, i * 128 : (i + 1) * 128],
                        rhs=wvt[m][:, j, :],
                        start=(j == 0),
                        stop=(j == 7),
                    )
                nc.vector.tensor_add(
                    vt[4 * c + i][:, 2 * m : 2 * m + 2, 0:64],
                    ps[:, 0:128].rearrange("p (h d) -> p h d", h=2),
                    bvb[:, m * 128 : (m + 1) * 128].rearrange("p (h d) -> p h d", h=2),
                )

            # ---- attention stream ----
            # groups h-major; the first two groups (h0/h1 at qb0) interleave
            # per k-block (each gets its own ctx bank via bufs=2).
            groups = [(h, qb) for h in range(8) for qb in range(4)]
            slots = []
            for kb in range(NKB):
                slots.append((0, 0, kb))
                slots.append((1, 0, kb))
            slots += [
                (h, qb, kb)
                for (h, qb) in groups
                if (h, qb) not in ((0, 0), (1, 0))
                for kb in range(NKB)
            ]
            NS = len(slots)

            # engine assignment: 3/8 of slots exp on DVE
            def is_dve(s):
                return s % 8 in (1, 3, 6)

            # K/Q projection units spread through the previous window so the
            # qkv psum rotation matches execution order. V projections are
            # emitted just-in-time in the attention band (ensure_v).
            kq_sched = {}

            def sched(s, item):
                kq_sched.setdefault(s, []).append(item)

            for c in range(1, 4):
                sched(2 + 4 * (c - 1), ("k", 0, c))
                sched(4 + 4 * (c - 1), ("q", 0, c))
            for m in range(1, 4):
                bs = 64 * (m - 1) + 16
                if m >= 2:
                    sched(bs - 4, ("wl", "k", m))
                    sched(bs - 2, ("wl", "q", m))
                sched(bs, ("kq", m, 0))
                for c in range(1, 4):
                    sched(bs + 8 * c - 4, ("k", m, c))
                    sched(bs + 8 * c, ("q", m, c))

            def emit_fill(s):
                for u in kq_sched.get(s, ()):
                    with band(fill_cur):
                        if u[0] == "wl":
                            load_w(u[1], u[2])
                        elif u[0] == "kq":
                            proj_kq_fused(u[1], u[2])
                        else:
                            proj_qk(u[0], u[1], u[2])

            emitted_v = set()

            def ensure_v(s):
                h, qb, kb = slots[s]
                m = h // 2
                if (m, kb) not in emitted_v:
                    emitted_v.add((m, kb))
                    with band(att_cur):
                        proj_v(m, kb // 4, kb % 4)

            def emit_scores(s):
                h, qb, kb = slots[s]
                m, p0 = h // 2, 64 * (h % 2)
                with band(att_cur):
                    sc = psum.tile([128, 512], f32, tag="sc", bufs=4, name="sc")
                    kb_ap = (
                        kt8[m][p0 : p0 + 64, kb * 128 : (kb + 1) * 128]
                        .rearrange("p (one f) -> p one f", one=1)
                        .broadcast_to([64, 2, 128])
                    )
                    nc.tensor.matmul(
                        sc,
                        lhsT=kb_ap,
                        rhs=qt8[m][p0 : p0 + 64, :, qb * 512 : (qb + 1) * 512],
                        start=True,
                        stop=True,
                        perf_mode=DR,
                    )
                return sc

            def emit_exp(s, sc):
                with band(att_cur):
                    if is_dve(s):
                        ed = epool.tile([128, 512], i16, tag="ed", bufs=6, name="ed")
                        nc.vector.tensor_scalar(
                            ed, sc, A_EXP, B_EXP, Alu.mult, Alu.add
                        )
                        return ed.bitcast(mybir.dt.bfloat16)
                    ee = epool.tile([128, 512], mybir.dt.bfloat16, tag="ee",
                                    bufs=8, name="ee")
                    nc.scalar.activation(ee, sc, Exp, scale=EXP_SCALE)
                    return ee

            ctx_of = {}

            def do_pv(ev, h, qb, kb):
                if kb == 0:
                    ctx_of[(h, qb)] = psum.tile(
                        [128, 4, 65], f32, tag="ctx", bufs=2, name="ctx"
                    )
                ctx = ctx_of[(h, qb)]
                for qt in range(4):
                    nc.tensor.matmul(
                        ctx[:, qt, :],
                        lhsT=ev[:, qt * 128 : (qt + 1) * 128],
                        rhs=vt[kb][:, h, :],
                        start=(kb == 0 and qt == 0),
                        stop=False,
                        skip_group_check=True,
                    )

            def do_norm(h, qb, t0, t1):
                ctx = ctx_of[(h, qb)]
                nt = t1 - t0
                rr = opool.tile([128, 4, 1], f32, tag="r", bufs=3, name="rr")
                nc.vector.reciprocal(rr[:, t0:t1], ctx[:, t0:t1, 64:65])
                ob = opool.tile([128, 4, 64], f32, tag="o", bufs=3, name="ob")
                nc.vector.tensor_mul(
                    ob[:, t0:t1], ctx[:, t0:t1, 0:64],
                    rr[:, t0:t1].broadcast_to([128, nt, 64]),
                )
                nc.sync.dma_start(
                    out=out_d[
                        qb * 512 + t0 * 128 : qb * 512 + t1 * 128,
                        h * 64 : (h + 1) * 64,
                    ].rearrange("(t p) d -> p t d", p=128),
                    in_=ob[:, t0:t1],
                )

            last_group = groups[-1]

            def emit_pv(s, ev):
                h, qb, kb = slots[s]
                with band(att_cur):
                    if kb == NKB - 1 and (h, qb) == last_group:
                        # split the final norm to shorten the tail
                        ctx = ctx_of[(h, qb)]
                        for qt in range(4):
                            nc.tensor.matmul(
                                ctx[:, qt, :],
                                lhsT=ev[:, qt * 128 : (qt + 1) * 128],
                                rhs=vt[kb][:, h, :],
                                start=False,
                                stop=False,
                                skip_group_check=True,
                            )
                            if qt == 1:
                                do_norm(h, qb, 0, 2)
                        do_norm(h, qb, 2, 4)
                        return
                    do_pv(ev, h, qb, kb)
                    if kb == NKB - 1:
                        do_norm(h, qb, 0, 4)

            with band(fill_cur):
                proj_kq_fused(0, 0)
            scs = {}
            for s in range(4):
                ensure_v(s)
                scs[s] = emit_scores(s)
            for s in range(NS):
                ev = emit_exp(s, scs.pop(s))
                if s + 4 < NS:
                    ensure_v(s + 4)
                    scs[s + 4] = emit_scores(s + 4)
                emit_pv(s, ev)
                emit_fill(s)

    nc.compile()
    return nc


def _get_nc():
    if "nc" not in _CACHE:
        _CACHE["nc"] = _build()
    return _CACHE["nc"]


def _in_maps(x, Wq, bq, Wk, bk, Wv, bv):
    import ml_dtypes

    bf = ml_dtypes.bfloat16
    x = np.asarray(x, np.float32)
    maps = []
    for c in range(NCORES):
        b, hh = c // 2, c % 2
        cs = slice(hh * COLS, (hh + 1) * COLS)

        def warr(W):
            # [1024, 512] -> [128 p, 4 m, 8 j, 128 c]
            a = np.asarray(W, np.float32)[:, cs].astype(bf)
            return np.ascontiguousarray(
                a.reshape(8, 128, 4, 128).transpose(1, 2, 0, 3)
            )

        xTr = x[b].T.astype(bf).reshape(8, 128, 4, 512).transpose(1, 2, 0, 3)
        # wv: [1024, 512] -> [4 m, 128 p, 8 j * 128 c]
        wvr = np.asarray(Wv, np.float32)[:, cs].astype(bf)
        wvr = wvr.reshape(8, 128, 4, 128).transpose(2, 1, 0, 3).reshape(4, 128, 1024)
        maps.append(
            {
                "xT": np.ascontiguousarray(xTr),
                "wq": warr(Wq),
                "wk": warr(Wk),
                "wv": np.ascontiguousarray(wvr),
                "bqk": np.ascontiguousarray(
                    np.concatenate(
                        [
                            np.asarray(bq, np.float32)[cs].reshape(4, 128).T,
                            np.asarray(bk, np.float32)[cs].reshape(4, 128).T,
                        ],
                        axis=1,
                    )
                ),
                "bv": np.ascontiguousarray(np.asarray(bv, np.float32)[cs]),
            }
        )
    return maps


def _run(inputs, trace=False):
    from concourse import bass_utils

    nc = _get_nc()
    res = bass_utils.run_bass_kernel_spmd(
        nc,
        _in_maps(**inputs),
        core_ids=list(range(NCORES)),
        trace=trace,
    )
    out = np.empty((B, S, D), np.float32)
    for c in range(NCORES):
        b, hh = c // 2, c % 2
        out[b, :, hh * COLS : (hh + 1) * COLS] = res.results[c]["out"]
    return out, res


def kernel(**inputs):
    out, _ = _run(inputs, trace=False)
    return out


if __name__ == "__main__":
    _get_nc()
    print("build ok")


# revision 6
# speedup vs baseline: 1.1702x; 1.0104x over previous
"""Trainium2 Bass kernel for nn_AttentionLayer (B=4, S=2048, H=16, DH=64).

Sharding: 8 cores = 4 batches x 2 head-halves. Core c handles batch c//2,
heads (c%2)*8 .. (c%2)*8+8 (512 of the 1024 QKV columns).

Design (v2, ~PE-bound at ~194us modeled):
  - Projections (PE, bf16): x[2048,1024] @ W[1024,512] per head-pair chunk,
    as in the baseline. Q/K write TRANSPOSED via the DVE epilogue.
  - Scores in fp8e4m3 with DoubleRow (0.5 cycles/row, halves score PE time):
    kt8[m] [128p=2 heads x 64dh, S] fp8 holds k8; qt8[m] [128, 2, S] holds
    (q8, qres8 = q - q8). The DoubleRow matmul contracts
    k8^T q8 + k8^T qres8 = k8^T q  -- the q-side fp8 quantization error is
    residual-corrected for free using the second k-tile slot (the k8 operand
    is a stride-0 broadcast AP, verified on HW). Only the k-side fp8 error
    (~2.5% elementwise) survives -> ~1% on probs.
  - exp is split between ACT and DVE. ACT units: activation(Exp, bf16 out).
    DVE units: ONE tensor_scalar t = s*A + B with int16 output, where
    A = 128*log2(e)/8 and B = 16256 - 128*0.0573. Round-to-nearest convert
    makes the int16 bit pattern EXACTLY the Schraudolph bf16 approximation
    of exp(s/8) (verified bit-exact on HW); PV reads the tile bitcast as
    bf16. ~1.7% RMS elementwise on the offloaded fraction only.
  - Score units are uniform 2-slot tiles (tag sc, [128,2,512], bufs=2 = 4
    PSUM banks) -> one exp instruction per 2 slots on either engine.
  - ctx accumulates per (head, qb) group in PSUM with bufs=2 (2 banks), so
    group boundaries do not serialize PE through the DVE norm. The first PV
    matmul of a group uses start=True to zero the whole bank (replaces the
    DVE memset). V tiles carry a ones-column so PV accumulates the softmax
    denominator in ctx col 64; norm = DVE reciprocal + broadcast mul.
  - PSUM banks: qkv 2 + sc 4 + ctx 2 = 8.
  - Startup: DMA order wk0,wq0,wv0,x0(4),x1(2),wv1,x2,x3,wv2,wv3,wk1,wq1
    (wv is loaded in per-pair chunks from a [4,128,1024] host layout so V
    for pair 0 is available right after x0). ~13 throwaway fp32 matmuls
    ramp the PE p-state during the DMA window.
  - The last group's norm/DMA is split in halves to shorten the tail.
"""

import numpy as np

B, S, H, DH = 4, 2048, 16, 64
D = H * DH  # 1024
NCORES = 8
COLS = 512  # qkv columns per core (8 heads)
NKB = 16
EXP_SCALE = 0.125  # 1/sqrt(DH)
LOG2E = 1.4426950408889634
A_EXP = 128.0 * LOG2E * 0.125
B_EXP = 16256.0 - 128.0 * 0.0573  # bf16 exp bias + Schraudolph centering

_CACHE = {}


def _build(n_dve_mod=(4, 3, 2)):
    import concourse.mybir as mybir
    import concourse.tile as tile
    from concourse import bacc

    f32 = mybir.dt.float32
    bf16 = mybir.dt.bfloat16
    fp8 = mybir.dt.float8e4
    i16 = mybir.dt.int16
    Alu = mybir.AluOpType
    Exp = mybir.ActivationFunctionType.Exp
    DR = mybir.MatmulPerfMode.DoubleRow

    nc = bacc.Bacc(
        "TRN2",
        target_bir_lowering=False,
        debug=False,
        enable_asserts=False,
        num_devices=NCORES,
    )

    xT_d = nc.dram_tensor("xT", [128, 4, 8, 512], bf16, kind="ExternalInput").ap()
    wq_d = nc.dram_tensor("wq", [128, 4, 8, 128], bf16, kind="ExternalInput").ap()
    wk_d = nc.dram_tensor("wk", [128, 4, 8, 128], bf16, kind="ExternalInput").ap()
    wv_d = nc.dram_tensor("wv", [4, 128, 1024], bf16, kind="ExternalInput").ap()
    bqk_d = nc.dram_tensor("bqk", [128, 8], f32, kind="ExternalInput").ap()
    bv_d = nc.dram_tensor("bv", [COLS], f32, kind="ExternalInput").ap()
    out_d = nc.dram_tensor("out", [S, COLS], f32, kind="ExternalOutput").ap()

    with tile.TileContext(nc) as tc:
        with (
            tc.tile_pool(name="consts", bufs=1) as consts,
            tc.tile_pool(name="wpool", bufs=1) as wpool,
            tc.tile_pool(name="qkp", bufs=1) as qkp,
            tc.tile_pool(name="vpool", bufs=1) as vpool,
            tc.tile_pool(name="xpool", bufs=1) as xpool,
            tc.tile_pool(name="epool", bufs=1) as epool,
            tc.tile_pool(name="opool", bufs=1) as opool,
            tc.tile_pool(name="psum", bufs=1, space="PSUM") as psum,
        ):
            from contextlib import contextmanager

            base = tc.cur_priority + 50
            att_cur = [base]
            fill_cur = [base + 8000]

            @contextmanager
            def band(cursor):
                off = tc.cur_priority - cursor[0]
                with tc.high_priority(offset=off):
                    yield
                    cursor[0] = tc.cur_priority

            # ---- constants + PE p-state warm-up ----
            with band(att_cur):
                warm = consts.tile([1, 1], f32)
                nc.vector.memset(warm, 0.0)
                nc.scalar.activation(warm, warm, Exp)  # pull ACT table load
                wsrc = consts.tile([128, 128], f32, name="wsrc")
                nc.vector.memset(wsrc, 0.0)
                for _ in range(13):
                    wps = psum.tile([128, 4, 65], f32, tag="ctx", bufs=2, name="wps")
                    nc.tensor.matmul(
                        wps.rearrange("p t d -> p (t d)")[:, 0:64],
                        lhsT=wsrc,
                        rhs=wsrc[:, 0:64],
                        start=True,
                        stop=True,
                    )

            with band(fill_cur):
                bqk_t = consts.tile([128, 8], f32)
                bv_s = consts.tile([1, COLS], f32)
                bvb = consts.tile([128, COLS], f32)
                nc.gpsimd.dma_start(out=bqk_t, in_=bqk_d)
                bq_t = bqk_t[:, 0:4]
                bk_t = bqk_t[:, 4:8]
                nc.gpsimd.dma_start(out=bv_s, in_=bv_d[None, :])
                nc.gpsimd.partition_broadcast(bvb, bv_s)

                vt = [vpool.tile([128, 8, 65], bf16, name=f"vt{i}") for i in range(NKB)]
                for i in range(NKB):
                    nc.vector.memset(vt[i][:, :, 64:65], 1.0)

                # fp8 q/k tiles: kt8[m] [128p = 2 heads x 64 dh, S];
                # qt8[m] [128, 2, S] = (q8, qres8)
                kt8 = [qkp.tile([128, S], fp8, name=f"kt8{m}") for m in range(4)]
                qt8 = [qkp.tile([128, 2, S], fp8, name=f"qt8{m}") for m in range(4)]
                wvt = [wpool.tile([128, 8, 128], bf16, name=f"wvt{m}") for m in range(4)]

            # ---- weight chunk ring ----
            wcur = {"q": {}, "k": {}}

            def load_w(proj, m):
                w_d = wq_d if proj == "q" else wk_d
                wt = wpool.tile(
                    [128, 8, 128], bf16, tag=f"w{proj}", bufs=2, name=f"w{proj}{m}"
                )
                nc.sync.dma_start(out=wt, in_=w_d[:, m, :, :])
                wcur[proj][m] = wt

            def load_wv(m):
                nc.sync.dma_start(
                    out=wvt[m], in_=wv_d[m].rearrange("p (j c) -> p j c", j=8)
                )

            with band(fill_cur):
                xt = [
                    xpool.tile([128, 8, 512], bf16, name=f"xt{c}") for c in range(4)
                ]

                # DMA order: startup-critical first; x0 split fine for
                # matmul chasing, later chunks coarser.
                load_w("k", 0)
                load_w("q", 0)
                for j0 in range(0, 8, 2):
                    nc.sync.dma_start(
                        out=xt[0][:, j0 : j0 + 2, :], in_=xT_d[:, 0, j0 : j0 + 2, :]
                    )
                load_wv(0)
                for j0 in range(0, 8, 4):
                    nc.sync.dma_start(
                        out=xt[1][:, j0 : j0 + 4, :], in_=xT_d[:, 1, j0 : j0 + 4, :]
                    )
                for j0 in range(0, 8, 4):
                    nc.sync.dma_start(
                        out=xt[2][:, j0 : j0 + 4, :], in_=xT_d[:, 2, j0 : j0 + 4, :]
                    )
                load_wv(1)
                nc.sync.dma_start(out=xt[3], in_=xT_d[:, 3, :, :])
                load_wv(2)
                load_wv(3)
                load_w("k", 1)
                load_w("q", 1)

            # ---- projection epilogues (DVE, PSUM -> fp8 SBUF) ----
            def epi_k(m, c, ps):
                ch = slice(c * 512, (c + 1) * 512)
                nc.vector.tensor_scalar_add(kt8[m][:, ch], ps, bk_t[:, m : m + 1])

            def epi_q(m, c, ps):
                ch = slice(c * 512, (c + 1) * 512)
                nc.vector.tensor_scalar_add(qt8[m][:, 0, ch], ps, bq_t[:, m : m + 1])
                nc.vector.scalar_tensor_tensor(
                    qt8[m][:, 1, ch],
                    ps,
                    bq_t[:, m : m + 1],
                    qt8[m][:, 0, ch],
                    Alu.add,
                    Alu.subtract,
                )

            # ---- projection unit emitters (fill band) ----
            def proj_kq_fused(m, c):
                psk = psum.tile([128, 512], f32, tag="qkv", bufs=2, name="psk")
                psq = psum.tile([128, 512], f32, tag="qkv", bufs=2, name="psq2")
                for j in range(8):
                    nc.tensor.matmul(
                        psk, lhsT=wcur["k"][m][:, j, :], rhs=xt[c][:, j, :],
                        start=(j == 0), stop=(j == 7),
                    )
                    nc.tensor.matmul(
                        psq, lhsT=wcur["q"][m][:, j, :], rhs=xt[c][:, j, :],
                        start=(j == 0), stop=(j == 7),
                    )
                epi_k(m, c, psk)
                epi_q(m, c, psq)

            def proj_qk(proj, m, c):
                w = wcur[proj][m]
                ps = psum.tile([128, 512], f32, tag="qkv", bufs=2, name="psq")
                for j in range(8):
                    nc.tensor.matmul(
                        ps,
                        lhsT=w[:, j, :],
                        rhs=xt[c][:, j, :],
                        start=(j == 0),
                        stop=(j == 7),
                    )
                (epi_q if proj == "q" else epi_k)(m, c, ps)

            def proj_v(m, c, i):
                ps = psum.tile([128, 512], f32, tag="qkv", bufs=2, name="psv")
                for j in range(8):
                    nc.tensor.matmul(
                        ps[:, 0:128],
                        lhsT=xt[c][:, j, i * 128 : (i + 1) * 128],
                        rhs=wvt[m][:, j, :],
                        start=(j == 0),
                        stop=(j == 7),
                    )
                nc.vector.tensor_add(
                    vt[4 * c + i][:, 2 * m : 2 * m + 2, 0:64],
                    ps[:, 0:128].rearrange("p (h d) -> p h d", h=2),
                    bvb[:, m * 128 : (m + 1) * 128].rearrange("p (h d) -> p h d", h=2),
                )

            # ---- attention stream ----
            # pair-0/1 groups h-major; pairs 2 and 3 interleave at group
            # granularity so the exp-heavy tail is shared by both pairs
            # (pair-3 slots start right after its projections land). The
            # first two groups (h0/h1 at qb0) interleave per k-block (each
            # gets its own ctx bank via bufs=2).
            groups = [(h, qb) for h in (0, 1, 2, 3) for qb in range(4)]
            for qb in range(4):
                groups += [(4, qb), (6, qb)]
            for qb in range(4):
                groups += [(5, qb), (7, qb)]
            slots = []
            for kb in range(NKB):
                slots.append((0, 0, kb))
                slots.append((1, 0, kb))
            slots += [
                (h, qb, kb)
                for (h, qb) in groups
                if (h, qb) not in ((0, 0), (1, 0))
                for kb in range(NKB)
            ]
            NS = len(slots)

            # engine assignment: 3/8 DVE in the proj-heavy first half,
            # ~7/16 in the balanced second half
            def is_dve(s):
                if s < 256:
                    return s % 8 in (1, 3, 6)
                return s % 16 in (0, 2, 4, 6, 8, 10, 13)

            # K/Q projection units spread through the previous window so the
            # qkv psum rotation matches execution order. V projections are
            # emitted just-in-time in the attention band (ensure_v).
            kq_sched = {}

            def sched(s, item):
                kq_sched.setdefault(s, []).append(item)

            for c in range(1, 4):
                sched(2 + 8 * (c - 1), ("k", 0, c))
                sched(6 + 8 * (c - 1), ("q", 0, c))
            PAIR_SCHED = {1: 30, 2: 136, 3: 210}
            for m in range(1, 4):
                bs = PAIR_SCHED[m]
                if m >= 2:
                    sched(bs - 16, ("wl", "k", m))
                    sched(bs - 14, ("wl", "q", m))
                sched(bs, ("kq", m, 0))
                for c in range(1, 4):
                    sched(bs + 14 * c - 6, ("k", m, c))
                    sched(bs + 14 * c, ("q", m, c))

            def emit_fill(s):
                for u in kq_sched.get(s, ()):
                    with band(fill_cur):
                        if u[0] == "wl":
                            load_w(u[1], u[2])
                        elif u[0] == "kq":
                            proj_kq_fused(u[1], u[2])
                        else:
                            proj_qk(u[0], u[1], u[2])

            emitted_v = set()

            def ensure_v(s):
                h, qb, kb = slots[s]
                m = h // 2
                if (m, kb) not in emitted_v:
                    emitted_v.add((m, kb))
                    with band(att_cur):
                        proj_v(m, kb // 4, kb % 4)

            def emit_scores(s):
                h, qb, kb = slots[s]
                m, p0 = h // 2, 64 * (h % 2)
                with band(att_cur):
                    sc = psum.tile([128, 512], f32, tag="sc", bufs=4, name="sc")
                    kb_ap = (
                        kt8[m][p0 : p0 + 64, kb * 128 : (kb + 1) * 128]
                        .rearrange("p (one f) -> p one f", one=1)
                        .broadcast_to([64, 2, 128])
                    )
                    nc.tensor.matmul(
                        sc,
                        lhsT=kb_ap,
                        rhs=qt8[m][p0 : p0 + 64, :, qb * 512 : (qb + 1) * 512],
                        start=True,
                        stop=True,
                        perf_mode=DR,
                    )
                return sc

            def emit_exp(s, sc):
                with band(att_cur):
                    if is_dve(s):
                        ed = epool.tile([128, 512], i16, tag="ed", bufs=6, name="ed")
                        nc.vector.tensor_scalar(
                            ed, sc, A_EXP, B_EXP, Alu.mult, Alu.add
                        )
                        return ed.bitcast(mybir.dt.bfloat16)
                    ee = epool.tile([128, 512], mybir.dt.bfloat16, tag="ee",
                                    bufs=8, name="ee")
                    nc.scalar.activation(ee, sc, Exp, scale=EXP_SCALE)
                    return ee

            ctx_of = {}

            def do_pv(ev, h, qb, kb):
                if kb == 0:
                    ctx_of[(h, qb)] = psum.tile(
                        [128, 4, 65], f32, tag="ctx", bufs=2, name="ctx"
                    )
                ctx = ctx_of[(h, qb)]
                for qt in range(4):
                    nc.tensor.matmul(
                        ctx[:, qt, :],
                        lhsT=ev[:, qt * 128 : (qt + 1) * 128],
                        rhs=vt[kb][:, h, :],
                        start=(kb == 0 and qt == 0),
                        stop=False,
                        skip_group_check=True,
                    )

            def do_norm(h, qb, t0, t1):
                ctx = ctx_of[(h, qb)]
                nt = t1 - t0
                rr = opool.tile([128, 4, 1], f32, tag="r", bufs=3, name="rr")
                nc.vector.reciprocal(rr[:, t0:t1], ctx[:, t0:t1, 64:65])
                ob = opool.tile([128, 4, 64], f32, tag="o", bufs=3, name="ob")
                nc.vector.tensor_mul(
                    ob[:, t0:t1], ctx[:, t0:t1, 0:64],
                    rr[:, t0:t1].broadcast_to([128, nt, 64]),
                )
                nc.sync.dma_start(
                    out=out_d[
                        qb * 512 + t0 * 128 : qb * 512 + t1 * 128,
                        h * 64 : (h + 1) * 64,
                    ].rearrange("(t p) d -> p t d", p=128),
                    in_=ob[:, t0:t1],
                )

            last_group = groups[-1]

            def emit_pv(s, ev):
                h, qb, kb = slots[s]
                with band(att_cur):
                    if kb == NKB - 1 and (h, qb) == last_group:
                        # split the final norm to shorten the tail
                        ctx = ctx_of[(h, qb)]
                        for qt in range(4):
                            nc.tensor.matmul(
                                ctx[:, qt, :],
                                lhsT=ev[:, qt * 128 : (qt + 1) * 128],
                                rhs=vt[kb][:, h, :],
                                start=False,
                                stop=False,
                                skip_group_check=True,
                            )
                            if qt == 1:
                                do_norm(h, qb, 0, 2)
                        do_norm(h, qb, 2, 4)
                        return
                    do_pv(ev, h, qb, kb)
                    if kb == NKB - 1:
                        do_norm(h, qb, 0, 4)

            with band(fill_cur):
                proj_kq_fused(0, 0)
            scs = {}
            for s in range(4):
                scs[s] = emit_scores(s)
            for s in range(4):
                ensure_v(s)
            for s in range(NS):
                ev = emit_exp(s, scs.pop(s))
                if s + 4 < NS:
                    ensure_v(s + 4)
                    scs[s + 4] = emit_scores(s + 4)
                emit_pv(s, ev)
                emit_fill(s)

    nc.compile()
    return nc


def _get_nc():
    if "nc" not in _CACHE:
        _CACHE["nc"] = _build()
    return _CACHE["nc"]


def _in_maps(x, Wq, bq, Wk, bk, Wv, bv):
    import ml_dtypes

    bf = ml_dtypes.bfloat16
    x = np.asarray(x, np.float32)
    maps = []
    for c in range(NCORES):
        b, hh = c // 2, c % 2
        cs = slice(hh * COLS, (hh + 1) * COLS)

        def warr(W):
            # [1024, 512] -> [128 p, 4 m, 8 j, 128 c]
            a = np.asarray(W, np.float32)[:, cs].astype(bf)
            return np.ascontiguousarray(
                a.reshape(8, 128, 4, 128).transpose(1, 2, 0, 3)
            )

        xTr = x[b].T.astype(bf).reshape(8, 128, 4, 512).transpose(1, 2, 0, 3)
        # wv: [1024, 512] -> [4 m, 128 p, 8 j * 128 c]
        wvr = np.asarray(Wv, np.float32)[:, cs].astype(bf)
        wvr = wvr.reshape(8, 128, 4, 128).transpose(2, 1, 0, 3).reshape(4, 128, 1024)
        maps.append(
            {
                "xT": np.ascontiguousarray(xTr),
                "wq": warr(Wq),
                "wk": warr(Wk),
                "wv": np.ascontiguousarray(wvr),
                "bqk": np.ascontiguousarray(
                    np.concatenate(
                        [
                            np.asarray(bq, np.float32)[cs].reshape(4, 128).T,
                            np.asarray(bk, np.float32)[cs].reshape(4, 128).T,
                        ],
                        axis=1,
                    )
                ),
                "bv": np.ascontiguousarray(np.asarray(bv, np.float32)[cs]),
            }
        )
    return maps


def _run(inputs, trace=False):
    from concourse import bass_utils

    nc = _get_nc()
    res = bass_utils.run_bass_kernel_spmd(
        nc,
        _in_maps(**inputs),
        core_ids=list(range(NCORES)),
        trace=trace,
    )
    out = np.empty((B, S, D), np.float32)
    for c in range(NCORES):
        b, hh = c // 2, c % 2
        out[b, :, hh * COLS : (hh + 1) * COLS] = res.results[c]["out"]
    return out, res


def kernel(**inputs):
    out, _ = _run(inputs, trace=False)
    return out


if __name__ == "__main__":
    _get_nc()
    print("build ok")


# revision 14
# speedup vs baseline: 1.2050x; 1.0297x over previous
"""Trainium2 Bass kernel for nn_AttentionLayer (B=4, S=2048, H=16, DH=64).

Sharding: 8 cores = 4 batches x 2 head-halves. Core c handles batch c//2,
heads (c%2)*8 .. (c%2)*8+8 (512 of the 1024 QKV columns).

Design (v2, ~PE-bound at ~194us modeled):
  - Projections (PE, bf16): x[2048,1024] @ W[1024,512] per head-pair chunk,
    as in the baseline. Q/K write TRANSPOSED via the DVE epilogue.
  - Scores in fp8e4m3 with DoubleRow (0.5 cycles/row, halves score PE time):
    kt8[m] [128p=2 heads x 64dh, S] fp8 holds k8; qt8[m] [128, 2, S] holds
    (q8, qres8 = q - q8). The DoubleRow matmul contracts
    k8^T q8 + k8^T qres8 = k8^T q  -- the q-side fp8 quantization error is
    residual-corrected for free using the second k-tile slot (the k8 operand
    is a stride-0 broadcast AP, verified on HW). Only the k-side fp8 error
    (~2.5% elementwise) survives -> ~1% on probs.
  - exp is split between ACT and DVE. ACT units: activation(Exp, bf16 out).
    DVE units: ONE tensor_scalar t = s*A + B with int16 output, where
    A = 128*log2(e)/8 and B = 16256 - 128*0.0573. Round-to-nearest convert
    makes the int16 bit pattern EXACTLY the Schraudolph bf16 approximation
    of exp(s/8) (verified bit-exact on HW); PV reads the tile bitcast as
    bf16. ~1.7% RMS elementwise on the offloaded fraction only.
  - Score units are uniform 2-slot tiles (tag sc, [128,2,512], bufs=2 = 4
    PSUM banks) -> one exp instruction per 2 slots on either engine.
  - ctx accumulates per (head, qb) group in PSUM with bufs=2 (2 banks), so
    group boundaries do not serialize PE through the DVE norm. The first PV
    matmul of a group uses start=True to zero the whole bank (replaces the
    DVE memset). V tiles carry a ones-column so PV accumulates the softmax
    denominator in ctx col 64; norm = DVE reciprocal + broadcast mul.
  - PSUM banks: qkv 2 + sc 4 + ctx 2 = 8.
  - Startup: DMA order wk0,wq0,wv0,x0(4),x1(2),wv1,x2,x3,wv2,wv3,wk1,wq1
    (wv is loaded in per-pair chunks from a [4,128,1024] host layout so V
    for pair 0 is available right after x0). ~13 throwaway fp32 matmuls
    ramp the PE p-state during the DMA window.
  - The last group's norm/DMA is split in halves to shorten the tail.
"""

import numpy as np

B, S, H, DH = 4, 2048, 16, 64
D = H * DH  # 1024
NCORES = 8
COLS = 512  # qkv columns per core (8 heads)
NKB = 16
EXP_SCALE = 0.125  # 1/sqrt(DH)
LOG2E = 1.4426950408889634
A_EXP = 128.0 * LOG2E * 0.125
B_EXP = 16256.0 - 128.0 * 0.0573  # bf16 exp bias + Schraudolph centering

_CACHE = {}


def _build(n_dve_mod=(4, 3, 2)):
    import concourse.mybir as mybir
    import concourse.tile as tile
    from concourse import bacc

    f32 = mybir.dt.float32
    bf16 = mybir.dt.bfloat16
    fp8 = mybir.dt.float8e4
    i16 = mybir.dt.int16
    Alu = mybir.AluOpType
    Exp = mybir.ActivationFunctionType.Exp
    DR = mybir.MatmulPerfMode.DoubleRow

    nc = bacc.Bacc(
        "TRN2",
        target_bir_lowering=False,
        debug=False,
        enable_asserts=False,
        num_devices=NCORES,
    )

    xT_d = nc.dram_tensor("xT", [128, 4, 8, 512], bf16, kind="ExternalInput").ap()
    wq_d = nc.dram_tensor("wq", [128, 4, 8, 128], bf16, kind="ExternalInput").ap()
    wk_d = nc.dram_tensor("wk", [128, 4, 8, 128], bf16, kind="ExternalInput").ap()
    wv_d = nc.dram_tensor("wv", [4, 128, 1024], bf16, kind="ExternalInput").ap()
    bqk_d = nc.dram_tensor("bqk", [128, 8], f32, kind="ExternalInput").ap()
    bv_d = nc.dram_tensor("bv", [COLS], f32, kind="ExternalInput").ap()
    out_d = nc.dram_tensor("out", [S, COLS], f32, kind="ExternalOutput").ap()

    with tile.TileContext(nc) as tc:
        with (
            tc.tile_pool(name="consts", bufs=1) as consts,
            tc.tile_pool(name="wpool", bufs=1) as wpool,
            tc.tile_pool(name="qkp", bufs=1) as qkp,
            tc.tile_pool(name="vpool", bufs=1) as vpool,
            tc.tile_pool(name="xpool", bufs=1) as xpool,
            tc.tile_pool(name="epool", bufs=1) as epool,
            tc.tile_pool(name="opool", bufs=1) as opool,
            tc.tile_pool(name="psum", bufs=1, space="PSUM") as psum,
        ):
            from contextlib import contextmanager

            base = tc.cur_priority + 50
            att_cur = [base]
            fill_cur = [base + 8000]

            @contextmanager
            def band(cursor):
                off = tc.cur_priority - cursor[0]
                with tc.high_priority(offset=off):
                    yield
                    cursor[0] = tc.cur_priority

            # ---- constants + PE p-state warm-up ----
            with band(att_cur):
                warm = consts.tile([1, 1], f32)
                nc.vector.memset(warm, 0.0)
                nc.scalar.activation(warm, warm, Exp)  # pull ACT table load
                wsrc = consts.tile([128, 128], f32, name="wsrc")
                nc.vector.memset(wsrc, 0.0)
                for _ in range(13):
                    wps = psum.tile([128, 4, 65], f32, tag="ctx", bufs=2, name="wps")
                    nc.tensor.matmul(
                        wps.rearrange("p t d -> p (t d)")[:, 0:64],
                        lhsT=wsrc,
                        rhs=wsrc[:, 0:64],
                        start=True,
                        stop=True,
                    )

            with band(fill_cur):
                bqk_t = consts.tile([128, 8], f32)
                bv_s = consts.tile([1, COLS], f32)
                bvb = consts.tile([128, COLS], f32)
                nc.gpsimd.dma_start(out=bqk_t, in_=bqk_d)
                bq_t = bqk_t[:, 0:4]
                bk_t = bqk_t[:, 4:8]
                nc.gpsimd.dma_start(out=bv_s, in_=bv_d[None, :])
                nc.gpsimd.partition_broadcast(bvb, bv_s)

                vt = vpool.tile([128, NKB, 8, 65], bf16, name="vt")
                nc.vector.memset(vt[:, :, :, 64:65], 1.0)

                # fp8 q/k tiles: kt8[m] [128p = 2 heads x 64 dh, S];
                # qt8[m] [128, 2, S] = (q8, qres8)
                kt8 = [qkp.tile([128, S], fp8, name=f"kt8{m}") for m in range(4)]
                qt8 = [qkp.tile([128, 2, S], fp8, name=f"qt8{m}") for m in range(4)]
                wvt = [wpool.tile([128, 8, 128], bf16, name=f"wvt{m}") for m in range(4)]

            # ---- weight chunk ring ----
            wcur = {"q": {}, "k": {}}

            def load_w(proj, m):
                w_d = wq_d if proj == "q" else wk_d
                wt = wpool.tile(
                    [128, 8, 128], bf16, tag=f"w{proj}", bufs=2, name=f"w{proj}{m}"
                )
                nc.sync.dma_start(out=wt, in_=w_d[:, m, :, :])
                wcur[proj][m] = wt

            def load_wv(m):
                nc.sync.dma_start(
                    out=wvt[m], in_=wv_d[m].rearrange("p (j c) -> p j c", j=8)
                )

            with band(fill_cur):
                xt = [
                    xpool.tile([128, 8, 512], bf16, name=f"xt{c}") for c in range(4)
                ]

                # DMA order: startup-critical first; x0 split fine for
                # matmul chasing, later chunks coarser.
                load_w("k", 0)
                load_w("q", 0)
                for j0 in range(0, 8, 2):
                    nc.sync.dma_start(
                        out=xt[0][:, j0 : j0 + 2, :], in_=xT_d[:, 0, j0 : j0 + 2, :]
                    )
                load_wv(0)
                for j0 in range(0, 8, 4):
                    nc.sync.dma_start(
                        out=xt[1][:, j0 : j0 + 4, :], in_=xT_d[:, 1, j0 : j0 + 4, :]
                    )
                for j0 in range(0, 8, 4):
                    nc.sync.dma_start(
                        out=xt[2][:, j0 : j0 + 4, :], in_=xT_d[:, 2, j0 : j0 + 4, :]
                    )
                load_wv(1)
                nc.sync.dma_start(out=xt[3], in_=xT_d[:, 3, :, :])
                load_wv(2)
                load_wv(3)
                load_w("k", 1)
                load_w("q", 1)

            # ---- projection epilogues (DVE, PSUM -> fp8 SBUF) ----
            def epi_k(m, c, ps):
                ch = slice(c * 512, (c + 1) * 512)
                nc.vector.tensor_scalar_add(kt8[m][:, ch], ps, bk_t[:, m : m + 1])

            def epi_q(m, c, ps):
                ch = slice(c * 512, (c + 1) * 512)
                nc.vector.tensor_scalar_add(qt8[m][:, 0, ch], ps, bq_t[:, m : m + 1])
                nc.vector.scalar_tensor_tensor(
                    qt8[m][:, 1, ch],
                    ps,
                    bq_t[:, m : m + 1],
                    qt8[m][:, 0, ch],
                    Alu.add,
                    Alu.subtract,
                )

            # ---- projection unit emitters (fill band) ----
            def proj_kq_fused(m, c):
                psk = psum.tile([128, 512], f32, tag="qkv", bufs=2, name="psk")
                psq = psum.tile([128, 512], f32, tag="qkv", bufs=2, name="psq2")
                for j in range(8):
                    nc.tensor.matmul(
                        psk, lhsT=wcur["k"][m][:, j, :], rhs=xt[c][:, j, :],
                        start=(j == 0), stop=(j == 7),
                    )
                    nc.tensor.matmul(
                        psq, lhsT=wcur["q"][m][:, j, :], rhs=xt[c][:, j, :],
                        start=(j == 0), stop=(j == 7),
                    )
                epi_k(m, c, psk)
                epi_q(m, c, psq)

            def proj_qk(proj, m, c):
                w = wcur[proj][m]
                ps = psum.tile([128, 512], f32, tag="qkv", bufs=2, name="psq")
                for j in range(8):
                    nc.tensor.matmul(
                        ps,
                        lhsT=w[:, j, :],
                        rhs=xt[c][:, j, :],
                        start=(j == 0),
                        stop=(j == 7),
                    )
                (epi_q if proj == "q" else epi_k)(m, c, ps)

            def proj_v(m, c):
                # all 4 seq-subchunks of chunk c in one psum tile + one
                # batched DVE add into the vt tile
                ps = psum.tile([128, 512], f32, tag="qkv", bufs=2, name="psv")
                for i in range(4):
                    for j in range(8):
                        nc.tensor.matmul(
                            ps[:, i * 128 : (i + 1) * 128],
                            lhsT=xt[c][:, j, i * 128 : (i + 1) * 128],
                            rhs=wvt[m][:, j, :],
                            # one start per bank-life: later sub-chunks'
                            # first writes consume the pending-zero bytes
                            start=(i == 0 and j == 0),
                            stop=(i == 3 and j == 7),
                            skip_group_check=True,
                        )
                nc.vector.tensor_add(
                    vt[:, 4 * c : 4 * c + 4, 2 * m : 2 * m + 2, 0:64],
                    ps.rearrange("p (i h d) -> p i h d", i=4, h=2),
                    bvb[:, m * 128 : (m + 1) * 128]
                    .rearrange("p (one h d) -> p one h d", one=1, h=2)
                    .broadcast_to([128, 4, 2, 64]),
                )

            # ---- attention stream ----
            # pair-0/1 groups h-major; pairs 2 and 3 interleave at group
            # granularity so the exp-heavy tail is shared by both pairs
            # (pair-3 slots start right after its projections land). The
            # first two groups (h0/h1 at qb0) interleave per k-block (each
            # gets its own ctx bank via bufs=2).
            groups = [(h, qb) for h in (0, 1, 2, 3) for qb in range(4)]
            for qb in range(4):
                groups += [(4, qb), (6, qb)]
            for qb in range(4):
                groups += [(5, qb), (7, qb)]
            slots = []
            for kb in range(NKB):
                slots.append((0, 0, kb))
                slots.append((1, 0, kb))
            slots += [
                (h, qb, kb)
                for (h, qb) in groups
                if (h, qb) not in ((0, 0), (1, 0))
                for kb in range(NKB)
            ]
            NS = len(slots)

            # engine assignment: 3/8 DVE in the proj-heavy first half,
            # ~7/16 in the balanced second half
            def is_dve(s):
                if s < 256:
                    return s % 8 in (1, 3, 6)
                return s % 16 in (0, 2, 4, 6, 8, 10, 13)

            # All projection units explicitly scheduled at slot positions so
            # the qkv psum rotation matches execution order. V quads for
            # pair m land shortly before pair-m attention consumes them.
            kq_sched = {}

            def sched(s, item):
                kq_sched.setdefault(s, []).append(item)

            for c in range(1, 4):
                sched(2 + 8 * (c - 1), ("k", 0, c))
                sched(6 + 8 * (c - 1), ("q", 0, c))
            for c in range(1, 4):  # v(0,0) is emitted in the prologue
                sched(4 * c - 2, ("v", 0, c))
            PAIR_SCHED = {1: 30, 2: 120, 3: 180}
            for m in range(1, 4):
                bs = PAIR_SCHED[m]
                if m >= 2:
                    sched(bs - 16, ("wl", "k", m))
                    sched(bs - 14, ("wl", "q", m))
                sched(bs, ("kq", m, 0))
                for c in range(1, 4):
                    sched(bs + 12 * c - 6, ("k", m, c))
                    sched(bs + 12 * c, ("q", m, c))
            # v quads: needed at pair-window start + 4*c slots
            for c in range(4):
                sched(104 + 5 * c, ("v", 1, c))
                sched(228 + 5 * c, ("v", 2, c))
                sched(250 + 4 * c, ("v", 3, c))

            def emit_fill(s):
                for u in kq_sched.get(s, ()):
                    with band(fill_cur):
                        if u[0] == "wl":
                            load_w(u[1], u[2])
                        elif u[0] == "kq":
                            proj_kq_fused(u[1], u[2])
                        elif u[0] == "v":
                            proj_v(u[1], u[2])
                        else:
                            proj_qk(u[0], u[1], u[2])

            def emit_scores(s):
                h, qb, kb = slots[s]
                m, p0 = h // 2, 64 * (h % 2)
                with band(att_cur):
                    sc = psum.tile([128, 512], f32, tag="sc", bufs=4, name="sc")
                    kb_ap = (
                        kt8[m][p0 : p0 + 64, kb * 128 : (kb + 1) * 128]
                        .rearrange("p (one f) -> p one f", one=1)
                        .broadcast_to([64, 2, 128])
                    )
                    nc.tensor.matmul(
                        sc,
                        lhsT=kb_ap,
                        rhs=qt8[m][p0 : p0 + 64, :, qb * 512 : (qb + 1) * 512],
                        start=True,
                        stop=True,
                        perf_mode=DR,
                    )
                return sc

            def emit_exp(s, sc):
                with band(att_cur):
                    if is_dve(s):
                        ed = epool.tile([128, 512], i16, tag="ed", bufs=6, name="ed")
                        nc.vector.tensor_scalar(
                            ed, sc, A_EXP, B_EXP, Alu.mult, Alu.add
                        )
                        return ed.bitcast(mybir.dt.bfloat16)
                    ee = epool.tile([128, 512], mybir.dt.bfloat16, tag="ee",
                                    bufs=8, name="ee")
                    nc.scalar.activation(ee, sc, Exp, scale=EXP_SCALE)
                    return ee

            ctx_of = {}

            def do_pv(ev, h, qb, kb):
                if kb == 0:
                    ctx_of[(h, qb)] = psum.tile(
                        [128, 4, 65], f32, tag="ctx", bufs=2, name="ctx"
                    )
                ctx = ctx_of[(h, qb)]
                for qt in range(4):
                    nc.tensor.matmul(
                        ctx[:, qt, :],
                        lhsT=ev[:, qt * 128 : (qt + 1) * 128],
                        rhs=vt[:, kb, h, :],
                        start=(kb == 0 and qt == 0),
                        stop=False,
                        skip_group_check=True,
                    )

            def do_norm(h, qb, t0, t1):
                ctx = ctx_of[(h, qb)]
                nt = t1 - t0
                rr = opool.tile([128, 4, 1], f32, tag="r", bufs=3, name="rr")
                nc.vector.reciprocal(rr[:, t0:t1], ctx[:, t0:t1, 64:65])
                ob = opool.tile([128, 4, 64], f32, tag="o", bufs=3, name="ob")
                nc.vector.tensor_mul(
                    ob[:, t0:t1], ctx[:, t0:t1, 0:64],
                    rr[:, t0:t1].broadcast_to([128, nt, 64]),
                )
                nc.sync.dma_start(
                    out=out_d[
                        qb * 512 + t0 * 128 : qb * 512 + t1 * 128,
                        h * 64 : (h + 1) * 64,
                    ].rearrange("(t p) d -> p t d", p=128),
                    in_=ob[:, t0:t1],
                )

            last_group = groups[-1]

            def emit_pv(s, ev):
                h, qb, kb = slots[s]
                with band(att_cur):
                    if kb == NKB - 1 and (h, qb) == last_group:
                        # split the final norm to shorten the tail
                        ctx = ctx_of[(h, qb)]
                        for qt in range(4):
                            nc.tensor.matmul(
                                ctx[:, qt, :],
                                lhsT=ev[:, qt * 128 : (qt + 1) * 128],
                                rhs=vt[:, kb, h, :],
                                start=False,
                                stop=False,
                                skip_group_check=True,
                            )
                            if qt == 1:
                                do_norm(h, qb, 0, 2)
                        do_norm(h, qb, 2, 4)
                        return
                    do_pv(ev, h, qb, kb)
                    if kb == NKB - 1:
                        do_norm(h, qb, 0, 4)

            with band(fill_cur):
                proj_kq_fused(0, 0)
            scs = {}
            for s in range(4):
                scs[s] = emit_scores(s)
            with band(fill_cur):
                proj_v(0, 0)
            for s in range(NS):
                ev = emit_exp(s, scs.pop(s))
                if s + 4 < NS:
                    scs[s + 4] = emit_scores(s + 4)
                emit_pv(s, ev)
                emit_fill(s)

    nc.compile()
    return nc


def _get_nc():
    if "nc" not in _CACHE:
        _CACHE["nc"] = _build()
    return _CACHE["nc"]


def _in_maps(x, Wq, bq, Wk, bk, Wv, bv):
    import ml_dtypes

    bf = ml_dtypes.bfloat16
    x = np.asarray(x, np.float32)
    maps = []
    for c in range(NCORES):
        b, hh = c // 2, c % 2
        cs = slice(hh * COLS, (hh + 1) * COLS)

        def warr(W):
            # [1024, 512] -> [128 p, 4 m, 8 j, 128 c]
            a = np.asarray(W, np.float32)[:, cs].astype(bf)
            return np.ascontiguousarray(
                a.reshape(8, 128, 4, 128).transpose(1, 2, 0, 3)
            )

        xTr = x[b].T.astype(bf).reshape(8, 128, 4, 512).transpose(1, 2, 0, 3)
        # wv: [1024, 512] -> [4 m, 128 p, 8 j * 128 c]
        wvr = np.asarray(Wv, np.float32)[:, cs].astype(bf)
        wvr = wvr.reshape(8, 128, 4, 128).transpose(2, 1, 0, 3).reshape(4, 128, 1024)
        maps.append(
            {
                "xT": np.ascontiguousarray(xTr),
                "wq": warr(Wq),
                "wk": warr(Wk),
                "wv": np.ascontiguousarray(wvr),
                "bqk": np.ascontiguousarray(
                    np.concatenate(
                        [
                            np.asarray(bq, np.float32)[cs].reshape(4, 128).T,
                            np.asarray(bk, np.float32)[cs].reshape(4, 128).T,
                        ],
                        axis=1,
                    )
                ),
                "bv": np.ascontiguousarray(np.asarray(bv, np.float32)[cs]),
            }
        )
    return maps


def _run(inputs, trace=False):
    from concourse import bass_utils

    nc = _get_nc()
    res = bass_utils.run_bass_kernel_spmd(
        nc,
        _in_maps(**inputs),
        core_ids=list(range(NCORES)),
        trace=trace,
    )
    out = np.empty((B, S, D), np.float32)
    for c in range(NCORES):
        b, hh = c // 2, c % 2
        out[b, :, hh * COLS : (hh + 1) * COLS] = res.results[c]["out"]
    return out, res


def kernel(**inputs):
    out, _ = _run(inputs, trace=False)
    return out


if __name__ == "__main__":
    _get_nc()
    print("build ok")


# revision 16
# speedup vs baseline: 1.2105x; 1.0046x over previous
"""Trainium2 Bass kernel for nn_AttentionLayer (B=4, S=2048, H=16, DH=64).

Sharding: 8 cores = 4 batches x 2 head-halves. Core c handles batch c//2,
heads (c%2)*8 .. (c%2)*8+8 (512 of the 1024 QKV columns).

Design (v2, ~PE-bound at ~194us modeled):
  - Projections (PE, bf16): x[2048,1024] @ W[1024,512] per head-pair chunk,
    as in the baseline. Q/K write TRANSPOSED via the DVE epilogue.
  - Scores in fp8e4m3 with DoubleRow (0.5 cycles/row, halves score PE time):
    kt8[m] [128p=2 heads x 64dh, S] fp8 holds k8; qt8[m] [128, 2, S] holds
    (q8, qres8 = q - q8). The DoubleRow matmul contracts
    k8^T q8 + k8^T qres8 = k8^T q  -- the q-side fp8 quantization error is
    residual-corrected for free using the second k-tile slot (the k8 operand
    is a stride-0 broadcast AP, verified on HW). Only the k-side fp8 error
    (~2.5% elementwise) survives -> ~1% on probs.
  - exp is split between ACT and DVE. ACT units: activation(Exp, bf16 out).
    DVE units: ONE tensor_scalar t = s*A + B with int16 output, where
    A = 128*log2(e)/8 and B = 16256 - 128*0.0573. Round-to-nearest convert
    makes the int16 bit pattern EXACTLY the Schraudolph bf16 approximation
    of exp(s/8) (verified bit-exact on HW); PV reads the tile bitcast as
    bf16. ~1.7% RMS elementwise on the offloaded fraction only.
  - Score units are uniform 2-slot tiles (tag sc, [128,2,512], bufs=2 = 4
    PSUM banks) -> one exp instruction per 2 slots on either engine.
  - ctx accumulates per (head, qb) group in PSUM with bufs=2 (2 banks), so
    group boundaries do not serialize PE through the DVE norm. The first PV
    matmul of a group uses start=True to zero the whole bank (replaces the
    DVE memset). V tiles carry a ones-column so PV accumulates the softmax
    denominator in ctx col 64; norm = DVE reciprocal + broadcast mul.
  - PSUM banks: qkv 2 + sc 4 + ctx 2 = 8.
  - Startup: DMA order wk0,wq0,wv0,x0(4),x1(2),wv1,x2,x3,wv2,wv3,wk1,wq1
    (wv is loaded in per-pair chunks from a [4,128,1024] host layout so V
    for pair 0 is available right after x0). ~13 throwaway fp32 matmuls
    ramp the PE p-state during the DMA window.
  - The last group's norm/DMA is split in halves to shorten the tail.
"""

import numpy as np

B, S, H, DH = 4, 2048, 16, 64
D = H * DH  # 1024
NCORES = 8
COLS = 512  # qkv columns per core (8 heads)
NKB = 16
EXP_SCALE = 0.125  # 1/sqrt(DH)
LOG2E = 1.4426950408889634
A_EXP = 128.0 * LOG2E * 0.125
B_EXP = 16256.0 - 128.0 * 0.0573  # bf16 exp bias + Schraudolph centering

_CACHE = {}


def _build(n_dve_mod=(4, 3, 2)):
    import concourse.mybir as mybir
    import concourse.tile as tile
    from concourse import bacc

    f32 = mybir.dt.float32
    bf16 = mybir.dt.bfloat16
    fp8 = mybir.dt.float8e4
    i16 = mybir.dt.int16
    Alu = mybir.AluOpType
    Exp = mybir.ActivationFunctionType.Exp
    DR = mybir.MatmulPerfMode.DoubleRow

    nc = bacc.Bacc(
        "TRN2",
        target_bir_lowering=False,
        debug=False,
        enable_asserts=False,
        num_devices=NCORES,
    )

    xT_d = nc.dram_tensor("xT", [128, 4, 8, 512], bf16, kind="ExternalInput").ap()
    wq_d = nc.dram_tensor("wq", [128, 4, 8, 128], bf16, kind="ExternalInput").ap()
    wk_d = nc.dram_tensor("wk", [128, 4, 8, 128], bf16, kind="ExternalInput").ap()
    wv_d = nc.dram_tensor("wv", [4, 128, 1024], bf16, kind="ExternalInput").ap()
    bqk_d = nc.dram_tensor("bqk", [128, 8], f32, kind="ExternalInput").ap()
    bv_d = nc.dram_tensor("bv", [COLS], f32, kind="ExternalInput").ap()
    out_d = nc.dram_tensor("out", [S, COLS], f32, kind="ExternalOutput").ap()

    with tile.TileContext(nc) as tc:
        with (
            tc.tile_pool(name="consts", bufs=1) as consts,
            tc.tile_pool(name="wpool", bufs=1) as wpool,
            tc.tile_pool(name="qkp", bufs=1) as qkp,
            tc.tile_pool(name="vpool", bufs=1) as vpool,
            tc.tile_pool(name="xpool", bufs=1) as xpool,
            tc.tile_pool(name="epool", bufs=1) as epool,
            tc.tile_pool(name="opool", bufs=1) as opool,
            tc.tile_pool(name="psum", bufs=1, space="PSUM") as psum,
        ):
            from contextlib import contextmanager

            base = tc.cur_priority + 50
            att_cur = [base]
            fill_cur = [base + 8000]

            @contextmanager
            def band(cursor):
                off = tc.cur_priority - cursor[0]
                with tc.high_priority(offset=off):
                    yield
                    cursor[0] = tc.cur_priority

            # ---- constants + PE p-state warm-up ----
            with band(att_cur):
                warm = consts.tile([1, 1], f32)
                nc.vector.memset(warm, 0.0)
                nc.scalar.activation(warm, warm, Exp)  # pull ACT table load
                wsrc = consts.tile([128, 128], f32, name="wsrc")
                nc.vector.memset(wsrc, 0.0)
                for _ in range(13):
                    wps = psum.tile([128, 4, 65], f32, tag="ctx", bufs=2, name="wps")
                    nc.tensor.matmul(
                        wps.rearrange("p t d -> p (t d)")[:, 0:64],
                        lhsT=wsrc,
                        rhs=wsrc[:, 0:64],
                        start=True,
                        stop=True,
                    )

            with band(fill_cur):
                bqk_t = consts.tile([128, 8], f32)
                bv_s = consts.tile([1, COLS], f32)
                bvb = consts.tile([128, COLS], f32)
                nc.gpsimd.dma_start(out=bqk_t, in_=bqk_d)
                bq_t = bqk_t[:, 0:4]
                bk_t = bqk_t[:, 4:8]
                nc.gpsimd.dma_start(out=bv_s, in_=bv_d[None, :])
                nc.gpsimd.partition_broadcast(bvb, bv_s)

                vt = vpool.tile([128, NKB, 8, 65], bf16, name="vt")
                nc.vector.memset(vt[:, :, :, 64:65], 1.0)

                # fp8 q/k tiles: kt8[m] [128p = 2 heads x 64 dh, S];
                # qt8[m] [128, 2, S] = (q8, qres8)
                kt8 = [qkp.tile([128, S], fp8, name=f"kt8{m}") for m in range(4)]
                qt8 = [qkp.tile([128, 2, S], fp8, name=f"qt8{m}") for m in range(4)]
                wvt = [wpool.tile([128, 8, 128], bf16, name=f"wvt{m}") for m in range(4)]

            # ---- weight chunk ring ----
            wcur = {"q": {}, "k": {}}

            def load_w(proj, m):
                w_d = wq_d if proj == "q" else wk_d
                wt = wpool.tile(
                    [128, 8, 128], bf16, tag=f"w{proj}", bufs=2, name=f"w{proj}{m}"
                )
                nc.sync.dma_start(out=wt, in_=w_d[:, m, :, :])
                wcur[proj][m] = wt

            def load_wv(m):
                nc.sync.dma_start(
                    out=wvt[m], in_=wv_d[m].rearrange("p (j c) -> p j c", j=8)
                )

            with band(fill_cur):
                xt = [
                    xpool.tile([128, 8, 512], bf16, name=f"xt{c}") for c in range(4)
                ]

                # DMA order: startup-critical first; x0 split fine for
                # matmul chasing, later chunks coarser.
                load_w("k", 0)
                load_w("q", 0)
                for j0 in range(0, 8, 2):
                    nc.sync.dma_start(
                        out=xt[0][:, j0 : j0 + 2, :], in_=xT_d[:, 0, j0 : j0 + 2, :]
                    )
                load_wv(0)
                for j0 in range(0, 8, 4):
                    nc.sync.dma_start(
                        out=xt[1][:, j0 : j0 + 4, :], in_=xT_d[:, 1, j0 : j0 + 4, :]
                    )
                for j0 in range(0, 8, 4):
                    nc.sync.dma_start(
                        out=xt[2][:, j0 : j0 + 4, :], in_=xT_d[:, 2, j0 : j0 + 4, :]
                    )
                load_wv(1)
                nc.sync.dma_start(out=xt[3], in_=xT_d[:, 3, :, :])
                load_wv(2)
                load_wv(3)
                load_w("k", 1)
                load_w("q", 1)

            # ---- projection epilogues (DVE, PSUM -> fp8 SBUF) ----
            def epi_k(m, c, ps):
                ch = slice(c * 512, (c + 1) * 512)
                nc.vector.tensor_scalar_add(kt8[m][:, ch], ps, bk_t[:, m : m + 1])

            def epi_q(m, c, ps):
                ch = slice(c * 512, (c + 1) * 512)
                nc.vector.tensor_scalar_add(qt8[m][:, 0, ch], ps, bq_t[:, m : m + 1])
                nc.vector.scalar_tensor_tensor(
                    qt8[m][:, 1, ch],
                    ps,
                    bq_t[:, m : m + 1],
                    qt8[m][:, 0, ch],
                    Alu.add,
                    Alu.subtract,
                )

            # ---- projection unit emitters (fill band) ----
            def proj_kq_fused(m, c):
                psk = psum.tile([128, 512], f32, tag="qkv", bufs=2, name="psk")
                psq = psum.tile([128, 512], f32, tag="qkv", bufs=2, name="psq2")
                for j in range(8):
                    nc.tensor.matmul(
                        psk, lhsT=wcur["k"][m][:, j, :], rhs=xt[c][:, j, :],
                        start=(j == 0), stop=(j == 7),
                    )
                    nc.tensor.matmul(
                        psq, lhsT=wcur["q"][m][:, j, :], rhs=xt[c][:, j, :],
                        start=(j == 0), stop=(j == 7),
                    )
                epi_k(m, c, psk)
                epi_q(m, c, psq)

            def proj_qk(proj, m, c):
                w = wcur[proj][m]
                ps = psum.tile([128, 512], f32, tag="qkv", bufs=2, name="psq")
                for j in range(8):
                    nc.tensor.matmul(
                        ps,
                        lhsT=w[:, j, :],
                        rhs=xt[c][:, j, :],
                        start=(j == 0),
                        stop=(j == 7),
                    )
                (epi_q if proj == "q" else epi_k)(m, c, ps)

            def proj_v(m, c):
                # all 4 seq-subchunks of chunk c in one psum tile + one
                # batched DVE add into the vt tile
                ps = psum.tile([128, 512], f32, tag="qkv", bufs=2, name="psv")
                for i in range(4):
                    for j in range(8):
                        nc.tensor.matmul(
                            ps[:, i * 128 : (i + 1) * 128],
                            lhsT=xt[c][:, j, i * 128 : (i + 1) * 128],
                            rhs=wvt[m][:, j, :],
                            # one start per bank-life: later sub-chunks'
                            # first writes consume the pending-zero bytes
                            start=(i == 0 and j == 0),
                            stop=(i == 3 and j == 7),
                            skip_group_check=True,
                        )
                nc.vector.tensor_add(
                    vt[:, 4 * c : 4 * c + 4, 2 * m : 2 * m + 2, 0:64],
                    ps.rearrange("p (i h d) -> p i h d", i=4, h=2),
                    bvb[:, m * 128 : (m + 1) * 128]
                    .rearrange("p (one h d) -> p one h d", one=1, h=2)
                    .broadcast_to([128, 4, 2, 64]),
                )

            # ---- attention stream ----
            # pair-0/1 groups h-major; pairs 2 and 3 interleave at group
            # granularity so the exp-heavy tail is shared by both pairs
            # (pair-3 slots start right after its projections land). The
            # first two groups (h0/h1 at qb0) interleave per k-block (each
            # gets its own ctx bank via bufs=2).
            groups = [(h, qb) for h in (0, 1, 2, 3) for qb in range(4)]
            # pairs 2/3 interleaved, pair-3 ramping in late enough that its
            # projections can run on the merged phase's PE slack
            groups += [(4, 0), (4, 1), (6, 0), (4, 2), (6, 1), (4, 3),
                       (6, 2), (5, 0), (6, 3), (5, 1), (7, 0), (5, 2),
                       (7, 1), (5, 3), (7, 2), (7, 3)]
            slots = []
            for kb in range(NKB):
                slots.append((0, 0, kb))
                slots.append((1, 0, kb))
            slots += [
                (h, qb, kb)
                for (h, qb) in groups
                if (h, qb) not in ((0, 0), (1, 0))
                for kb in range(NKB)
            ]
            NS = len(slots)

            # engine assignment: 3/8 DVE in the proj-heavy first half,
            # ~7/16 in the balanced second half
            def is_dve(s):
                if s < 256:
                    return s % 8 in (1, 3, 6)
                return s % 16 in (0, 2, 4, 6, 8, 10, 13)

            # All projection units explicitly scheduled at slot positions so
            # the qkv psum rotation matches execution order. V quads for
            # pair m land shortly before pair-m attention consumes them.
            kq_sched = {}

            def sched(s, item):
                kq_sched.setdefault(s, []).append(item)

            for c in range(1, 4):
                sched(2 + 8 * (c - 1), ("k", 0, c))
                sched(6 + 8 * (c - 1), ("q", 0, c))
            for c in range(1, 4):  # v(0,0) is emitted in the prologue
                sched(4 * c - 2, ("v", 0, c))
            PAIR_SCHED = {1: 30, 2: 136, 3: 252}
            for m in range(1, 4):
                bs = PAIR_SCHED[m]
                if m >= 2:
                    sched(bs - 16, ("wl", "k", m))
                    sched(bs - 14, ("wl", "q", m))
                sched(bs, ("kq", m, 0))
                for c in range(1, 4):
                    sched(bs + 8 * c - 4, ("k", m, c))
                    sched(bs + 8 * c, ("q", m, c))
            # v quads: needed at pair-window start + 4*c slots
            for c in range(4):
                sched(104 + 5 * c, ("v", 1, c))
                sched(228 + 5 * c, ("v", 2, c))
                sched(278 + 3 * c, ("v", 3, c))

            def emit_fill(s):
                for u in kq_sched.get(s, ()):
                    with band(fill_cur):
                        if u[0] == "wl":
                            load_w(u[1], u[2])
                        elif u[0] == "kq":
                            proj_kq_fused(u[1], u[2])
                        elif u[0] == "v":
                            proj_v(u[1], u[2])
                        else:
                            proj_qk(u[0], u[1], u[2])

            def emit_scores(s):
                h, qb, kb = slots[s]
                m, p0 = h // 2, 64 * (h % 2)
                with band(att_cur):
                    sc = psum.tile([128, 512], f32, tag="sc", bufs=4, name="sc")
                    kb_ap = (
                        kt8[m][p0 : p0 + 64, kb * 128 : (kb + 1) * 128]
                        .rearrange("p (one f) -> p one f", one=1)
                        .broadcast_to([64, 2, 128])
                    )
                    nc.tensor.matmul(
                        sc,
                        lhsT=kb_ap,
                        rhs=qt8[m][p0 : p0 + 64, :, qb * 512 : (qb + 1) * 512],
                        start=True,
                        stop=True,
                        perf_mode=DR,
                    )
                return sc

            def emit_exp(s, sc):
                with band(att_cur):
                    if is_dve(s):
                        ed = epool.tile([128, 512], i16, tag="ed", bufs=6, name="ed")
                        nc.vector.tensor_scalar(
                            ed, sc, A_EXP, B_EXP, Alu.mult, Alu.add
                        )
                        return ed.bitcast(mybir.dt.bfloat16)
                    ee = epool.tile([128, 512], mybir.dt.bfloat16, tag="ee",
                                    bufs=8, name="ee")
                    nc.scalar.activation(ee, sc, Exp, scale=EXP_SCALE)
                    return ee

            ctx_of = {}

            def do_pv(ev, h, qb, kb):
                if kb == 0:
                    ctx_of[(h, qb)] = psum.tile(
                        [128, 4, 65], f32, tag="ctx", bufs=2, name="ctx"
                    )
                ctx = ctx_of[(h, qb)]
                for qt in range(4):
                    nc.tensor.matmul(
                        ctx[:, qt, :],
                        lhsT=ev[:, qt * 128 : (qt + 1) * 128],
                        rhs=vt[:, kb, h, :],
                        start=(kb == 0 and qt == 0),
                        stop=False,
                        skip_group_check=True,
                    )

            def do_norm(h, qb, t0, t1):
                ctx = ctx_of[(h, qb)]
                nt = t1 - t0
                rr = opool.tile([128, 4, 1], f32, tag="r", bufs=3, name="rr")
                nc.vector.reciprocal(rr[:, t0:t1], ctx[:, t0:t1, 64:65])
                ob = opool.tile([128, 4, 64], f32, tag="o", bufs=3, name="ob")
                nc.vector.tensor_mul(
                    ob[:, t0:t1], ctx[:, t0:t1, 0:64],
                    rr[:, t0:t1].broadcast_to([128, nt, 64]),
                )
                nc.sync.dma_start(
                    out=out_d[
                        qb * 512 + t0 * 128 : qb * 512 + t1 * 128,
                        h * 64 : (h + 1) * 64,
                    ].rearrange("(t p) d -> p t d", p=128),
                    in_=ob[:, t0:t1],
                )

            last_group = groups[-1]

            def emit_pv(s, ev):
                h, qb, kb = slots[s]
                with band(att_cur):
                    if kb == NKB - 1 and (h, qb) == last_group:
                        # split the final norm to shorten the tail
                        ctx = ctx_of[(h, qb)]
                        for qt in range(4):
                            nc.tensor.matmul(
                                ctx[:, qt, :],
                                lhsT=ev[:, qt * 128 : (qt + 1) * 128],
                                rhs=vt[:, kb, h, :],
                                start=False,
                                stop=False,
                                skip_group_check=True,
                            )
                            if qt == 1:
                                do_norm(h, qb, 0, 2)
                        do_norm(h, qb, 2, 4)
                        return
                    do_pv(ev, h, qb, kb)
                    if kb == NKB - 1:
                        do_norm(h, qb, 0, 4)

            with band(fill_cur):
                proj_kq_fused(0, 0)
            scs = {}
            for s in range(4):
                scs[s] = emit_scores(s)
            with band(fill_cur):
                proj_v(0, 0)
            for s in range(NS):
                ev = emit_exp(s, scs.pop(s))
                if s + 4 < NS:
                    scs[s + 4] = emit_scores(s + 4)
                emit_pv(s, ev)
                emit_fill(s)

    nc.compile()
    return nc


def _get_nc():
    if "nc" not in _CACHE:
        _CACHE["nc"] = _build()
    return _CACHE["nc"]


def _in_maps(x, Wq, bq, Wk, bk, Wv, bv):
    import ml_dtypes

    bf = ml_dtypes.bfloat16
    x = np.asarray(x, np.float32)
    maps = []
    for c in range(NCORES):
        b, hh = c // 2, c % 2
        cs = slice(hh * COLS, (hh + 1) * COLS)

        def warr(W):
            # [1024, 512] -> [128 p, 4 m, 8 j, 128 c]
            a = np.asarray(W, np.float32)[:, cs].astype(bf)
            return np.ascontiguousarray(
                a.reshape(8, 128, 4, 128).transpose(1, 2, 0, 3)
            )

        xTr = x[b].T.astype(bf).reshape(8, 128, 4, 512).transpose(1, 2, 0, 3)
        # wv: [1024, 512] -> [4 m, 128 p, 8 j * 128 c]
        wvr = np.asarray(Wv, np.float32)[:, cs].astype(bf)
        wvr = wvr.reshape(8, 128, 4, 128).transpose(2, 1, 0, 3).reshape(4, 128, 1024)
        maps.append(
            {
                "xT": np.ascontiguousarray(xTr),
                "wq": warr(Wq),
                "wk": warr(Wk),
                "wv": np.ascontiguousarray(wvr),
                "bqk": np.ascontiguousarray(
                    np.concatenate(
                        [
                            np.asarray(bq, np.float32)[cs].reshape(4, 128).T,
                            np.asarray(bk, np.float32)[cs].reshape(4, 128).T,
                        ],
                        axis=1,
                    )
                ),
                "bv": np.ascontiguousarray(np.asarray(bv, np.float32)[cs]),
            }
        )
    return maps


def _run(inputs, trace=False):
    from concourse import bass_utils

    nc = _get_nc()
    res = bass_utils.run_bass_kernel_spmd(
        nc,
        _in_maps(**inputs),
        core_ids=list(range(NCORES)),
        trace=trace,
    )
    out = np.empty((B, S, D), np.float32)
    for c in range(NCORES):
        b, hh = c // 2, c % 2
        out[b, :, hh * COLS : (hh + 1) * COLS] = res.results[c]["out"]
    return out, res


def kernel(**inputs):
    out, _ = _run(inputs, trace=False)
    return out


if __name__ == "__main__":
    _get_nc()
    print("build ok")
